# revision 32
# baseline (speedup 1.0000x reference)
"""Trainium2 Bass kernel for PVT-style spatial-reduction attention.

Model (see reference):
  q = (x @ Wq + bq) * hd^-0.5                       (B, N, C) -> heads of 32
  x_ = BN(DWConv2x2s2(x)) ; k = x_ @ Wk + bk ; v = x_ @ Wv + bv
  attn = softmax(q k^T + rel_pos) ; out = (attn @ v) @ Wp + bp

Shapes: B=8, N=3136 (56x56), C=128, heads=4, hd=32, Nkv=784 (28x28).

Distribution: data-parallel over batch -- core j handles batch j fully
(B == n_cores == 8).  k/v/conv-taps are computed once per core (vs 8x
redundantly under query-sharding), cutting TensorE work by a third; the
exp(rel_pos) table is streamed per n-chunk from HBM instead.

Device layout strategy: features-on-partitions everywhere (C == 128).
  - host passes xT (B, C, N) in bf16; all projections are lhsT=weight
    matmuls.
  - conv+BN+k/v projection fused into 4 "tap" weight matrices (host
    precomputed); k-bias dropped (softmax-invariant), v-bias folded into
    the final bias.
  - scores computed transposed: S^T[m, n] per (nch, h), scaled by
    K = 128/ln2 on the weight side.  Softmax numerator per (r, hp) piece:
      hp==0 pieces: fused Schraudolph fast-exp -- ONE DVE tensor_add of a
        host-built int16 table round(K*R + 16256 - C) onto the f32 psum
        scores; the int16 sum IS the bf16 bit pattern of exp(S+R)
        (max ~4% per-weight error, geometric mean calibrated to 1 via C;
        net output error ~1.3e-3).  Also releases the scores psum tile
        fast, which the next round's pair-2 matmuls wait on.
      hp==1 pieces: exact ScalarE exp(S) (scale=1/K) then multiply by the
        bf16 exp(R) table on DVE or GpSimd (dense flat APs -> 2x DVE mode).
  - per (n-chunk, kv-chunk): attn@v runs all 4 heads 4-way column-packed
    (32-wide) into ONE psum bank; row sums via 4 column-packed all-ones
    matmuls (32-replicated) into a second bank, which doubles as the
    softmax-denominator broadcast: extract = reciprocal + multiply only.
  - the last slot accumulates attn@v in a retired scores-rotation psum
    tile so its chunks start inside the steady loop (shorter drain).
  - one-DMA weight load; biases on the sync ring; exp tables on the sync
    ring (scalar-ring backpressure stalled the q-bias IDENTITYs).
  - software pipeline per n-chunk slot (14 half-round steps):
      all steps:   scores+exp+mul of slot (kv chunk step//2, pair step%2)
      even steps:  attn@v + rowsums of slot-1, one kv chunk per step
      step 13:     extract (normalize) of slot-1
      step 0:      projection tail of slot-2
      step 1:      q projection of slot+1; expTI prefetch of slot+2
  - final output is produced transposed (C, N) per core; the host
    untransposes while assembling the full (B, N, C) result.
"""

import os
import sys

import numpy as np

if "/opt/trn_rl_repo" not in sys.path:
    sys.path.insert(0, "/opt/trn_rl_repo")

B = 8
N = 3136
C = 128
HEADS = 4
HD = 32
SR = 2
H = W = 56
NKV = 784  # 28*28
NCORES = 8
NSL = 448          # query rows per n-chunk slot (448 <= 512 psum bank)
NCH = N // NSL     # 7 slots per core
BN_EPS = 1e-5
SCALE = HD ** -0.5

# m (kv index) chunking: 784 = 6*128 + 16
M_CHUNKS = [(j * 128, min(128, NKV - j * 128)) for j in range((NKV + 127) // 128)]

PROB_BF16 = os.environ.get("KERNEL_PROB_BF16", "1") == "1"


def _parse_pieces(env, default):
    v = os.environ.get(env)
    if not v:
        return default
    return {tuple(int(x) for x in p.split(",")) for p in v.split(";") if p}


# (r, hp) half-rounds computed via fused Schraudolph fast-exp on DVE
# (exp(S+R) ~= bf16-bits of int16(K*(S+R) + B - C)); the rest use exact
# ScalarE exp + table multiply.  All hp==0 pieces use the fast path so the
# scores psum-tile rotation is released by a single quick DVE op (the
# next scores round's pair-2 waits on it); measured best balance:
# Scalar ~60us, DVE ~80us, GpSimd ~41us vs TensorE ~100us (the pacer).
SCH_PIECES = _parse_pieces(
    "KERNEL_SCH", {(r, 0) for r in range(7)})
# exact pieces whose exp(R) multiply runs on GpSimd instead of DVE.
# NOTE: 5-element GPS sets produced hardware-only corruption (see log);
# keep <= 4 pieces here.
GPS_MULS = _parse_pieces("KERNEL_GPS", {(0, 1), (2, 1), (4, 1)}) \
    - SCH_PIECES
K_SCH = 2.0 ** 7 / np.log(2.0)          # 184.665 (exp -> bf16-bit scale)
B_SCH = 127.0 * 128.0                   # bf16 exponent bias in bit units
C_SCH = float(os.environ.get("KERNEL_SCH_C", "7.333"))
if not PROB_BF16:
    SCH_PIECES = set()  # int16 bit-pun needs the bf16 table container

_COMPILED = None  # cached nc across kernel() calls
_PREP_CACHE = {}  # host-prep results cached by input id


def _host_prep(x, relative_pos, Wq, bq, Wk, bk, Wv, bv, conv_w, conv_b,
               bn_gamma, bn_beta, bn_mean, bn_var, Wp, bp):
    """Fuse conv/BN into tap weights; fold biases; transpose activations."""
    import ml_dtypes
    f32 = np.float32
    bf16 = ml_dtypes.bfloat16
    wdt = bf16 if PROB_BF16 else f32
    x = np.asarray(x, f32)
    # xT: (B, C, N)
    xT = np.ascontiguousarray(x.transpose(0, 2, 1).astype(wdt))

    inv = (np.asarray(bn_gamma, f32)
           / np.sqrt(np.asarray(bn_var, f32) + BN_EPS))          # [c]
    wp_taps = np.asarray(conv_w, f32).reshape(C, SR * SR) * inv[:, None]  # [c,4]
    beta0 = (np.asarray(conv_b, f32) * inv
             + np.asarray(bn_beta, f32)
             - np.asarray(bn_mean, f32) * inv)                    # [c]

    Wk = np.asarray(Wk, f32)
    Wv = np.asarray(Wv, f32)
    # Wk_tap[t, c, c'] = wp_taps[c, t] * Wk[c, c']
    Wk_tap = np.ascontiguousarray(
        (wp_taps.T[:, :, None] * Wk[None, :, :]).astype(wdt))     # (4, C, C)
    Wv_tap = np.ascontiguousarray(
        (wp_taps.T[:, :, None] * Wv[None, :, :]).astype(wdt))

    # v bias (uniform over kv positions -> exact fold into final bias)
    beta_v = beta0 @ Wv + np.asarray(bv, f32)                     # [c']
    bp_col = (np.asarray(bp, f32) + beta_v @ np.asarray(Wp, f32)).reshape(C, 1)

    # scores are computed as S*K_SCH on device (Wq pre-scaled); the exact
    # pieces undo it inside the activation (scale=1/K_SCH), the schraudolph
    # pieces consume it directly.
    Wq_s = np.ascontiguousarray(
        (np.asarray(Wq, f32) * (SCALE * K_SCH)).astype(wdt))
    bq_col = (np.asarray(bq, f32) * (SCALE * K_SCH)).reshape(C, 1)

    # rel^T table interleaved per n-chunk: (NCH, C, 7, HEADS, NSL).
    # Exact pieces (r, hp) store exp(R) in bf16; schraudolph pieces store
    # int16(K*R + B - C) bit-punned into the same bf16 container.
    rel = np.asarray(relative_pos, f32)                  # (4, N, NKV)
    rT = rel.transpose(0, 2, 1)                          # (4, NKV, N) f32
    expI = np.zeros((NCH, C, 7, HEADS, NSL), wdt)
    for j, (m0, cnt) in enumerate(M_CHUNKS):
        # (4, cnt, NCH, NSL) -> (NCH, cnt, h, NSL)
        blk = rT[:, m0:m0 + cnt, :].reshape(HEADS, cnt, NCH, NSL)
        blk = blk.transpose(2, 1, 0, 3)                  # (NCH, cnt, h, NSL)
        for hp in range(2):
            part = blk[:, :, 2 * hp:2 * hp + 2, :]
            if (j, hp) in SCH_PIECES:
                t = np.round(part * K_SCH + (B_SCH - C_SCH))
                dst = t.astype(np.int16).view(wdt)
            else:
                dst = np.exp(part).astype(wdt)
            expI[:, 0:cnt, j, 2 * hp:2 * hp + 2, :] = dst
    expI = np.ascontiguousarray(expI)

    # concat all bf16 weights into one [C, 10C] tensor (one DMA):
    # wq | wk taps (4C, interleaved t-major per c) | wv taps | wp
    Wp_c = np.asarray(Wp, f32).astype(wdt)
    wall = np.concatenate(
        [Wq_s,
         Wk_tap.transpose(1, 0, 2).reshape(C, 4 * C),
         Wv_tap.transpose(1, 0, 2).reshape(C, 4 * C),
         Wp_c], axis=1)
    ball = np.concatenate([bq_col, bp_col], axis=1)  # [C, 2] f32

    return dict(xT=xT, Wall=np.ascontiguousarray(wall),
                ball=np.ascontiguousarray(ball), expI=expI)


def _build():
    """Build + compile the SPMD bass program (same NEFF for all 8 cores)."""
    import concourse.bass as bass
    import concourse.tile as tile
    from concourse import bacc, mybir
    from concourse.masks import make_identity

    f32 = mybir.dt.float32
    f32r = mybir.dt.float32r
    pdt = mybir.dt.bfloat16 if PROB_BF16 else f32

    nc = bacc.Bacc("TRN2", target_bir_lowering=False, debug=False,
                   num_devices=NCORES)

    # ---- DRAM I/O ----
    xT_d = nc.dram_tensor("xT", [C, N], pdt, kind="ExternalInput").ap()
    expI_d = nc.dram_tensor("expI", [NCH, C, 7 * HEADS * NSL], pdt,
                            kind="ExternalInput").ap()
    Wall_d = nc.dram_tensor("Wall", [C, 10 * C], pdt,
                            kind="ExternalInput").ap()
    ball_d = nc.dram_tensor("ball", [C, 2], f32, kind="ExternalInput").ap()
    out_d = nc.dram_tensor("out", [C, N], f32, kind="ExternalOutput").ap()

    with tile.TileContext(nc) as tc:
        from contextlib import ExitStack
        with ExitStack() as ctx:
            _emit(ctx, tc, nc, bass, mybir, make_identity, f32, f32r, pdt,
                  xT_d, expI_d, Wall_d, ball_d, out_d)

    nc.compile()
    return nc


def _emit(ctx, tc, nc, bass, mybir, make_identity, f32, f32r, pdt,
          xT_d, expI_d, Wall_d, ball_d, out_d):
    AF = mybir.ActivationFunctionType

    singles = ctx.enter_context(tc.tile_pool(name="singles", bufs=1))
    ppool = ctx.enter_context(tc.tile_pool(name="ppool", bufs=3))
    epool = ctx.enter_context(tc.tile_pool(name="epool", bufs=3))
    opool = ctx.enter_context(tc.tile_pool(name="opool", bufs=3))
    qpool = ctx.enter_context(tc.tile_pool(name="qpool", bufs=3))
    ptpool = ctx.enter_context(tc.tile_pool(name="ptpool", bufs=6))
    # PSUM: rot 3x2 + out 1 + rs 1 = 8 banks
    ps_rot = ctx.enter_context(tc.tile_pool(name="ps_rot", bufs=3,
                                            space="PSUM"))
    ps_out = ctx.enter_context(tc.tile_pool(name="ps_out", bufs=1,
                                            space="PSUM"))
    ps_rs = ctx.enter_context(tc.tile_pool(name="ps_rs", bufs=1,
                                           space="PSUM"))

    # ---- constants ----
    identb = singles.tile([C, C], pdt)
    make_identity(nc, identb[:])
    ones_sb = singles.tile([C, HD], pdt)
    nc.vector.memset(ones_sb[:], 1.0)

    # all weights in ONE DMA (the ~700ns/DMA sequencer issue cost was
    # serializing the fill); biases in a second small one.
    wall_sb = singles.tile([C, 10 * C], pdt)
    nc.scalar.dma_start(out=wall_sb[:], in_=Wall_d)
    # biases ride the (quiet) sync ring so their completion isn't lumped
    # behind the exp-table loads on the scalar ring
    ball_sb = singles.tile([C, 2], f32)
    nc.sync.dma_start(out=ball_sb[:], in_=ball_d)
    wq_sb = wall_sb[:, 0:C]
    wk_sb = wall_sb[:, C:5 * C].rearrange("p (t d) -> p t d", t=4)
    wv_sb = wall_sb[:, 5 * C:9 * C].rearrange("p (t d) -> p t d", t=4)
    wp_sb = wall_sb[:, 9 * C:10 * C]
    bq_sb = ball_sb[:, 0:1]
    bp_sb = ball_sb[:, 1:2]

    # whole-batch activations + k/v, resident all kernel
    xT_sb = singles.tile([C, N], pdt)
    nc.sync.dma_start(out=xT_sb[:, 0:N // 2], in_=xT_d[:, 0:N // 2])
    nc.sync.dma_start(out=xT_sb[:, N // 2:N], in_=xT_d[:, N // 2:N])
    kT_sb = singles.tile([C, 7 * 128], pdt)
    nc.vector.memset(kT_sb[:, NKV:7 * 128], 0.0)
    vT_sb = singles.tile([C, NKV], pdt)
    v_sb = singles.tile([C, 7, HEADS, HD], pdt)

    state = {}
    pp_of = {}
    exp_of = {}

    def prep_tap(which, mc):
        """One kv-chunk of the fused conv-tap projection (4 matmuls)."""
        dst = kT_sb if which == 0 else vT_sb
        w_sb = wk_sb if which == 0 else wv_sb
        xview = xT_sb[:].rearrange("p (i a j c) -> p a c i j",
                                   i=28, a=2, j=28, c=2)
        ps_kv = ps_rot.tile([C, 2, 512], f32, tag="rot", name="ps_kv")
        ps_kv = ps_kv[:, 0, :]
        for t in range(SR * SR):
            di, dj = t // 2, t % 2
            rhs = xview[:, di, dj, 14 * mc:14 * mc + 14, :]
            nc.tensor.matmul(ps_kv[:, 0:392], lhsT=w_sb[:, t, :],
                             rhs=rhs, start=(t == 0), stop=(t == 3))
        nc.vector.tensor_copy(dst[:, 392 * mc:392 * (mc + 1)], ps_kv[:, 0:392])

    def prep_vtrans(j):
        m0, cnt = M_CHUNKS[j]
        ps_t = ps_rot.tile([C, 2, 512], pdt, tag="rot", name="ps_t")
        ps_t = ps_t[:, 0, :]
        nc.tensor.transpose(ps_t[0:cnt, 0:C], vT_sb[:, m0:m0 + cnt],
                            identb[:])
        nc.vector.tensor_copy(
            v_sb[0:cnt, j, :, :],
            ps_t[0:cnt, 0:C].rearrange("p (h d) -> p h d", h=HEADS, d=HD))

    def exp_load(nch, eng=None):
        """Prefetch the exp(rel) interleave for slot nch (4 DMAs: finer
        completion granularity lets early pieces start before the whole
        2.8MB table lands)."""
        eng = eng if eng is not None else nc.sync
        e_sb = epool.tile([C, 7, HEADS, NSL], pdt, tag="expTI", name="e_sb")
        exp_of[nch] = e_sb
        flat = e_sb[:].rearrange("p a h n -> p (a h n)")
        tot = 7 * HEADS * NSL
        qtr = tot // 4
        for i in range(4):
            eng.dma_start(out=flat[:, i * qtr:(i + 1) * qtr],
                          in_=expI_d[nch, :, i * qtr:(i + 1) * qtr])

    qT_sb = singles.tile([C, N], pdt)

    def prep_q(qc):
        """Whole-batch q projection, one slot's columns at fill time."""
        ps_q = ps_rot.tile([C, 2, 512], f32, tag="rot", name="ps_q")
        c0 = qc * NSL
        nc.tensor.matmul(ps_q[:, 0, 0:NSL], lhsT=wq_sb,
                         rhs=xT_sb[:, c0:c0 + NSL],
                         start=True, stop=True)
        nc.scalar.activation(qT_sb[:, c0:c0 + NSL],
                             ps_q[:, 0, 0:NSL], AF.Identity, bias=bq_sb)

    sco_of = {}

    def scores_pair(g, hp):
        """One head-pair of round g's score matmuls (2-way row packing).
        Pair 1 waits on the previous round's schraudolph ADD releasing its
        psum tile; emitting it one step later than pair 0 lets the ready
        attn@v/rowsum groups run during that wait instead of queuing
        behind it in the strict-FIFO Tensor queue."""
        if g >= NCH * 7:
            return
        nch, r = g // 7, g % 7
        ps_s = ps_rot.tile([C, 2, 512], f32, tag="rot", name="ps_s")
        sco_of.setdefault(g, [None, None])[hp] = ps_s
        for h in (2 * hp, 2 * hp + 1):
            nc.tensor.matmul(
                ps_s[0:128, h % 2, 0:NSL],
                lhsT=kT_sb[HD * h:HD * (h + 1), 128 * r:128 * (r + 1)],
                rhs=qT_sb[HD * h:HD * (h + 1), nch * NSL:(nch + 1) * NSL],
                start=True, stop=True,
                tile_position=(HD * h, 0))

    def scores_round(g):
        scores_pair(g, 0)
        scores_pair(g, 1)

    i16 = mybir.dt.int16

    def exp_mul(nch, r, hp):
        """exp(S+R) for chunk r, head pair hp: either exact (ScalarE exp of
        S*K/K then DVE/GpSimd multiply by the exp(R) table) or fused
        schraudolph (one DVE add of the int16 R-table onto S*K in PSUM,
        int16 result bits == bf16 exp)."""
        g = nch * 7 + r
        ps_s = sco_of[g][hp]
        if hp == 1:
            del sco_of[g]
        pslice = pp_of[nch][:, r, 2 * hp:2 * hp + 2, :]
        eslice = exp_of[nch][:, r, 2 * hp:2 * hp + 2, :]
        if (r, hp) in SCH_PIECES:
            nc.vector.tensor_add(pslice.bitcast(i16), ps_s[:, :, 0:NSL],
                                 eslice.bitcast(i16))
            return
        pt_sb = ptpool.tile([C, 2 * NSL], pdt, tag="pt")
        nc.scalar.activation(pt_sb[:], ps_s[:, :, 0:NSL], AF.Exp,
                             scale=1.0 / K_SCH)
        eng = nc.gpsimd if (r, hp) in GPS_MULS else nc.vector
        off = (r * HEADS + 2 * hp) * NSL
        ppf = pp_of[nch][:].rearrange("p a h n -> p (a h n)")
        ef = exp_of[nch][:].rearrange("p a h n -> p (a h n)")
        eng.tensor_mul(ppf[:, off:off + 2 * NSL], pt_sb[:],
                       ef[:, off:off + 2 * NSL])

    def attnv4(nch, r, part=None):
        """attn@v + rowsums for kv chunk r: all 4 heads column-packed.
        part='av' emits only attn@v, 'rs' only rowsums (lets the two groups
        land on different pipeline steps). The last slot accumulates in a
        retired scores-rotation tile so its rounds can start before the
        previous slot's extract frees ov/z."""
        s = state[nch]
        m0, cnt = M_CHUNKS[r]
        if r == 0 and "ov" not in s:
            if nch == NCH - 1:
                ovz = ps_rot.tile([C, 2, 512], f32, tag="rot", name="ps_ovz")
                s["ov"] = ovz[:, 0, :]
                s["z"] = ovz[:, 1, :]
            else:
                s["ov"] = ps_out.tile([C, 512], f32, tag="out", name="ps_ov")
                s["z"] = ps_rs.tile([C, 512], f32, tag="rs", name="ps_z")
        ps_ov, ps_z = s["ov"], s["z"]
        pp = pp_of[nch]
        if part in (None, "av"):
            for h in range(HEADS):
                nc.tensor.matmul(
                    ps_ov[HD * h:HD * (h + 1), 0:NSL],
                    lhsT=v_sb[0:cnt, r, h, :],
                    rhs=pp[0:cnt, r, h, :],
                    start=(r == 0), stop=(r == len(M_CHUNKS) - 1),
                    tile_position=(0, HD * h), skip_group_check=True)
        if part in (None, "rs"):
            for h in range(HEADS):
                nc.tensor.matmul(
                    ps_z[HD * h:HD * (h + 1), 0:NSL],
                    lhsT=ones_sb[0:cnt, :],
                    rhs=pp[0:cnt, r, h, :],
                    start=(r == 0), stop=(r == len(M_CHUNKS) - 1),
                    tile_position=(0, HD * h), skip_group_check=True)

    def extract(nch):
        """Normalize straight out of PSUM: recip(rowsums), multiply."""
        s = state[nch]
        ps_ov = s.pop("ov")
        ps_z = s.pop("z")
        rb_sb = opool.tile([C, NSL], f32, tag="rb")
        nc.vector.reciprocal_approx_fast(rb_sb[:], ps_z[0:C, 0:NSL])
        outT_sb = opool.tile([C, NSL], pdt, tag="outT")
        s["outT"] = outT_sb
        nc.vector.tensor_mul(outT_sb[:], ps_ov[0:C, 0:NSL], rb_sb[:])

    def proj_tail(nch):
        """Final projection in transposed layout; host untransposes."""
        s = state[nch]
        ps_ft = ps_rot.tile([C, 2, 512], f32, tag="rot", name="ps_ft")
        ps_ft = ps_ft[:, 0, :]
        nc.tensor.matmul(ps_ft[0:C, 0:NSL], lhsT=wp_sb,
                         rhs=s.pop("outT")[:], start=True, stop=True)
        fin_sb = opool.tile([C, NSL], f32, tag="fin")
        # bias-add on ScalarE: DVE is loaded with schraudolph/mul work
        nc.scalar.activation(fin_sb[:], ps_ft[0:C, 0:NSL],
                             AF.Identity, bias=bp_sb)
        nc.sync.dma_start(out=out_d[:, nch * NSL:(nch + 1) * NSL],
                          in_=fin_sb[:])
        state.pop(nch)
        pp_of.pop(nch, None)
        exp_of.pop(nch, None)

    # ---- fill: k/v + all of q once, first exp tables ----
    # exp tables ride the sync ring: the scalar sequencer must stay free to
    # issue the q-bias IDENTITYs (ring backpressure from 2.8MB table DMAs
    # otherwise stalls everything queued behind them on that engine).
    exp_load(0, nc.sync)
    # k-taps + first q slot gate scores_round(0); v-taps don't (only
    # vtrans needs them, much later) -- keep them off the critical path
    for mc in range(2):
        prep_tap(0, mc)
    prep_q(0)
    scores_round(0)
    for mc in range(2):
        prep_tap(1, mc)
    for qc in range(1, NCH):
        prep_q(qc)
    for j in range(7):
        prep_vtrans(j)
    exp_load(1, nc.sync)
    # ---- steady loop over n-chunk slots ----
    for nch in range(NCH):
        pp_of[nch] = ppool.tile([C, 7, HEADS, NSL], pdt, tag="pp",
                                name="pp_sb")
        state.setdefault(nch, {})
        for step in range(14):
            r, hp = step // 2, step % 2
            scores_pair(nch * 7 + r + 1, hp)
            exp_mul(nch, r, hp)
            if nch >= 1 and hp == 0 and step <= 12:
                attnv4(nch - 1, step // 2)
                if step == 12:
                    # extract right after the last kv chunk: gives the DVE
                    # recip+mul a one-step head start so proj_tail's matmul
                    # (next slot, step 0) doesn't stall the Tensor queue
                    extract(nch - 1)
            if nch == NCH - 1 and step >= 12:
                attnv4(nch, step - 12)
            if nch >= 2 and step == 0:
                proj_tail(nch - 2)
            if step == 1 and nch + 2 < NCH:
                exp_load(nch + 2)
    # drain
    proj_tail(NCH - 2)
    for r in range(2, 7):
        attnv4(NCH - 1, r)
    extract(NCH - 1)
    proj_tail(NCH - 1)


def _get_compiled():
    global _COMPILED
    if _COMPILED is None:
        _COMPILED = _build()
    return _COMPILED


def make_in_map(prep, j):
    return {
        "xT": np.ascontiguousarray(prep["xT"][j]),
        "expI": prep["expI"].reshape(NCH, C, 7 * HEADS * NSL),
        "Wall": prep["Wall"], "ball": prep["ball"],
    }


def kernel(x, relative_pos, Wq, bq, Wk, bk, Wv, bv, conv_w, conv_b,
           bn_gamma, bn_beta, bn_mean, bn_var, Wp, bp, H=56, W=56,
           _trace=False):
    from concourse.bass_utils import run_bass_kernel_spmd

    prep = _host_prep(x, relative_pos, Wq, bq, Wk, bk, Wv, bv, conv_w,
                      conv_b, bn_gamma, bn_beta, bn_mean, bn_var, Wp, bp)
    nc = _get_compiled()

    in_maps = [make_in_map(prep, j) for j in range(NCORES)]

    res = run_bass_kernel_spmd(nc, in_maps, core_ids=list(range(NCORES)),
                               trace=_trace)

    out = np.empty((B, N, C), np.float32)
    for j in range(NCORES):
        out[j] = res.results[j]["out"].T
    if _trace:
        kernel._last_result = res
    return out



# revision 34
# speedup vs baseline: 1.0087x; 1.0087x over previous
"""Trainium2 Bass kernel for PVT-style spatial-reduction attention.

Model (see reference):
  q = (x @ Wq + bq) * hd^-0.5                       (B, N, C) -> heads of 32
  x_ = BN(DWConv2x2s2(x)) ; k = x_ @ Wk + bk ; v = x_ @ Wv + bv
  attn = softmax(q k^T + rel_pos) ; out = (attn @ v) @ Wp + bp

Shapes: B=8, N=3136 (56x56), C=128, heads=4, hd=32, Nkv=784 (28x28).

Distribution: data-parallel over batch -- core j handles batch j fully
(B == n_cores == 8).  k/v/conv-taps are computed once per core (vs 8x
redundantly under query-sharding), cutting TensorE work by a third; the
exp(rel_pos) table is streamed per n-chunk from HBM instead.

Device layout strategy: features-on-partitions everywhere (C == 128).
  - host passes xT (B, C, N) in bf16; all projections are lhsT=weight
    matmuls.
  - conv+BN+k/v projection fused into 4 "tap" weight matrices (host
    precomputed); k-bias dropped (softmax-invariant), v-bias folded into
    the final bias.
  - scores computed transposed: S^T[m, n] per (nch, h), scaled by
    K = 128/ln2 on the weight side.  Softmax numerator per (r, hp) piece:
      hp==0 pieces: fused Schraudolph fast-exp -- ONE DVE tensor_add of a
        host-built int16 table round(K*R + 16256 - C) onto the f32 psum
        scores; the int16 sum IS the bf16 bit pattern of exp(S+R)
        (max ~4% per-weight error, geometric mean calibrated to 1 via C;
        net output error ~1.3e-3).  Also releases the scores psum tile
        fast, which the next round's pair-2 matmuls wait on.
      hp==1 pieces: exact ScalarE exp(S) (scale=1/K) then multiply by the
        bf16 exp(R) table on DVE or GpSimd (dense flat APs -> 2x DVE mode).
  - per (n-chunk, kv-chunk): attn@v runs all 4 heads 4-way column-packed
    (32-wide) into ONE psum bank; row sums via 4 column-packed all-ones
    matmuls (32-replicated) into a second bank, which doubles as the
    softmax-denominator broadcast: extract = reciprocal + multiply only.
  - the last slot accumulates attn@v in a retired scores-rotation psum
    tile so its chunks start inside the steady loop (shorter drain).
  - one-DMA weight load; biases on the sync ring; exp tables on the sync
    ring (scalar-ring backpressure stalled the q-bias IDENTITYs).
  - software pipeline per n-chunk slot (14 half-round steps):
      all steps:   scores+exp+mul of slot (kv chunk step//2, pair step%2)
      even steps:  attn@v + rowsums of slot-1, one kv chunk per step
      step 13:     extract (normalize) of slot-1
      step 0:      projection tail of slot-2
      step 1:      q projection of slot+1; expTI prefetch of slot+2
  - final output is produced transposed (C, N) per core; the host
    untransposes while assembling the full (B, N, C) result.
"""

import os
import sys

import numpy as np

if "/opt/trn_rl_repo" not in sys.path:
    sys.path.insert(0, "/opt/trn_rl_repo")

B = 8
N = 3136
C = 128
HEADS = 4
HD = 32
SR = 2
H = W = 56
NKV = 784  # 28*28
NCORES = 8
NSL = 448          # query rows per n-chunk slot (448 <= 512 psum bank)
NCH = N // NSL     # 7 slots per core
BN_EPS = 1e-5
SCALE = HD ** -0.5

# m (kv index) chunking: 784 = 6*128 + 16
M_CHUNKS = [(j * 128, min(128, NKV - j * 128)) for j in range((NKV + 127) // 128)]

PROB_BF16 = os.environ.get("KERNEL_PROB_BF16", "1") == "1"


def _parse_pieces(env, default):
    v = os.environ.get(env)
    if not v:
        return default
    return {tuple(int(x) for x in p.split(",")) for p in v.split(";") if p}


# (r, hp) half-rounds computed via fused Schraudolph fast-exp on DVE
# (exp(S+R) ~= bf16-bits of int16(K*(S+R) + B - C)); the rest use exact
# ScalarE exp + table multiply.  All hp==0 pieces use the fast path so the
# scores psum-tile rotation is released by a single quick DVE op (the
# next scores round's pair-2 waits on it); measured best balance:
# Scalar ~60us, DVE ~80us, GpSimd ~41us vs TensorE ~100us (the pacer).
SCH_PIECES = _parse_pieces(
    "KERNEL_SCH", {(r, 0) for r in range(7)})
# exact pieces whose exp(R) multiply runs on GpSimd instead of DVE.
# NOTE: 5-element GPS sets produced hardware-only corruption (see log);
# keep <= 4 pieces here.
GPS_MULS = _parse_pieces("KERNEL_GPS", {(0, 1), (2, 1), (4, 1)}) \
    - SCH_PIECES
K_SCH = 2.0 ** 7 / np.log(2.0)          # 184.665 (exp -> bf16-bit scale)
B_SCH = 127.0 * 128.0                   # bf16 exponent bias in bit units
C_SCH = float(os.environ.get("KERNEL_SCH_C", "7.333"))
if not PROB_BF16:
    SCH_PIECES = set()  # int16 bit-pun needs the bf16 table container

_COMPILED = None  # cached nc across kernel() calls
_PREP_CACHE = {}  # host-prep results cached by input id


def _host_prep(x, relative_pos, Wq, bq, Wk, bk, Wv, bv, conv_w, conv_b,
               bn_gamma, bn_beta, bn_mean, bn_var, Wp, bp):
    """Fuse conv/BN into tap weights; fold biases; transpose activations."""
    import ml_dtypes
    f32 = np.float32
    bf16 = ml_dtypes.bfloat16
    wdt = bf16 if PROB_BF16 else f32
    x = np.asarray(x, f32)
    # xT: (B, C, N)
    xT = np.ascontiguousarray(x.transpose(0, 2, 1).astype(wdt))

    inv = (np.asarray(bn_gamma, f32)
           / np.sqrt(np.asarray(bn_var, f32) + BN_EPS))          # [c]
    wp_taps = np.asarray(conv_w, f32).reshape(C, SR * SR) * inv[:, None]  # [c,4]
    beta0 = (np.asarray(conv_b, f32) * inv
             + np.asarray(bn_beta, f32)
             - np.asarray(bn_mean, f32) * inv)                    # [c]

    Wk = np.asarray(Wk, f32)
    Wv = np.asarray(Wv, f32)
    # Wk_tap[t, c, c'] = wp_taps[c, t] * Wk[c, c']
    Wk_tap = np.ascontiguousarray(
        (wp_taps.T[:, :, None] * Wk[None, :, :]).astype(wdt))     # (4, C, C)
    Wv_tap = np.ascontiguousarray(
        (wp_taps.T[:, :, None] * Wv[None, :, :]).astype(wdt))

    # v bias (uniform over kv positions -> exact fold into final bias)
    beta_v = beta0 @ Wv + np.asarray(bv, f32)                     # [c']
    bp_col = (np.asarray(bp, f32) + beta_v @ np.asarray(Wp, f32)).reshape(C, 1)

    # scores are computed as S*K_SCH on device (Wq pre-scaled); the exact
    # pieces undo it inside the activation (scale=1/K_SCH), the schraudolph
    # pieces consume it directly.
    Wq_s = np.ascontiguousarray(
        (np.asarray(Wq, f32) * (SCALE * K_SCH)).astype(wdt))
    bq_col = (np.asarray(bq, f32) * (SCALE * K_SCH)).reshape(C, 1)

    # rel^T table interleaved per n-chunk: (NCH, C, 7, HEADS, NSL).
    # Exact pieces (r, hp) store exp(R) in bf16; schraudolph pieces store
    # int16(K*R + B - C) bit-punned into the same bf16 container.
    rel = np.asarray(relative_pos, f32)                  # (4, N, NKV)
    rT = rel.transpose(0, 2, 1)                          # (4, NKV, N) f32
    expI = np.zeros((NCH, C, 7, HEADS, NSL), wdt)
    for j, (m0, cnt) in enumerate(M_CHUNKS):
        # (4, cnt, NCH, NSL) -> (NCH, cnt, h, NSL)
        blk = rT[:, m0:m0 + cnt, :].reshape(HEADS, cnt, NCH, NSL)
        blk = blk.transpose(2, 1, 0, 3)                  # (NCH, cnt, h, NSL)
        for hp in range(2):
            part = blk[:, :, 2 * hp:2 * hp + 2, :]
            if (j, hp) in SCH_PIECES:
                t = np.round(part * K_SCH + (B_SCH - C_SCH))
                dst = t.astype(np.int16).view(wdt)
            else:
                dst = np.exp(part).astype(wdt)
            expI[:, 0:cnt, j, 2 * hp:2 * hp + 2, :] = dst
    expI = np.ascontiguousarray(expI)

    # concat all bf16 weights into one [C, 10C] tensor (one DMA):
    # wq | wk taps (4C, interleaved t-major per c) | wv taps | wp
    Wp_c = np.asarray(Wp, f32).astype(wdt)
    wall = np.concatenate(
        [Wq_s,
         Wk_tap.transpose(1, 0, 2).reshape(C, 4 * C),
         Wv_tap.transpose(1, 0, 2).reshape(C, 4 * C),
         Wp_c], axis=1)
    ball = np.concatenate([bq_col, bp_col], axis=1)  # [C, 2] f32

    return dict(xT=xT, Wall=np.ascontiguousarray(wall),
                ball=np.ascontiguousarray(ball), expI=expI)


def _build():
    """Build + compile the SPMD bass program (same NEFF for all 8 cores)."""
    import concourse.bass as bass
    import concourse.tile as tile
    from concourse import bacc, mybir
    from concourse.masks import make_identity

    f32 = mybir.dt.float32
    f32r = mybir.dt.float32r
    pdt = mybir.dt.bfloat16 if PROB_BF16 else f32

    nc = bacc.Bacc("TRN2", target_bir_lowering=False, debug=False,
                   num_devices=NCORES)

    # ---- DRAM I/O ----
    xT_d = nc.dram_tensor("xT", [C, N], pdt, kind="ExternalInput").ap()
    expI_d = nc.dram_tensor("expI", [NCH, C, 7 * HEADS * NSL], pdt,
                            kind="ExternalInput").ap()
    Wall_d = nc.dram_tensor("Wall", [C, 10 * C], pdt,
                            kind="ExternalInput").ap()
    ball_d = nc.dram_tensor("ball", [C, 2], f32, kind="ExternalInput").ap()
    out_d = nc.dram_tensor("out", [C, N], f32, kind="ExternalOutput").ap()

    with tile.TileContext(nc) as tc:
        from contextlib import ExitStack
        with ExitStack() as ctx:
            _emit(ctx, tc, nc, bass, mybir, make_identity, f32, f32r, pdt,
                  xT_d, expI_d, Wall_d, ball_d, out_d)

    nc.compile()
    return nc


def _emit(ctx, tc, nc, bass, mybir, make_identity, f32, f32r, pdt,
          xT_d, expI_d, Wall_d, ball_d, out_d):
    AF = mybir.ActivationFunctionType

    singles = ctx.enter_context(tc.tile_pool(name="singles", bufs=1))
    ppool = ctx.enter_context(tc.tile_pool(name="ppool", bufs=3))
    epool = ctx.enter_context(tc.tile_pool(name="epool", bufs=3))
    opool = ctx.enter_context(tc.tile_pool(name="opool", bufs=3))
    qpool = ctx.enter_context(tc.tile_pool(name="qpool", bufs=3))
    ptpool = ctx.enter_context(tc.tile_pool(name="ptpool", bufs=6))
    # PSUM: rot 3x2 + out 1 + rs 1 = 8 banks
    ps_rot = ctx.enter_context(tc.tile_pool(name="ps_rot", bufs=3,
                                            space="PSUM"))
    ps_out = ctx.enter_context(tc.tile_pool(name="ps_out", bufs=1,
                                            space="PSUM"))
    ps_rs = ctx.enter_context(tc.tile_pool(name="ps_rs", bufs=1,
                                           space="PSUM"))

    # ---- constants ----
    identb = singles.tile([C, C], pdt)
    make_identity(nc, identb[:])
    ones_sb = singles.tile([C, HD], pdt)
    nc.vector.memset(ones_sb[:], 1.0)

    # all weights in ONE DMA (the ~700ns/DMA sequencer issue cost was
    # serializing the fill); biases in a second small one.
    wall_sb = singles.tile([C, 10 * C], pdt)
    nc.scalar.dma_start(out=wall_sb[:], in_=Wall_d)
    # biases ride the (quiet) sync ring so their completion isn't lumped
    # behind the exp-table loads on the scalar ring
    ball_sb = singles.tile([C, 2], f32)
    nc.sync.dma_start(out=ball_sb[:], in_=ball_d)
    wq_sb = wall_sb[:, 0:C]
    wk_sb = wall_sb[:, C:5 * C].rearrange("p (t d) -> p t d", t=4)
    wv_sb = wall_sb[:, 5 * C:9 * C].rearrange("p (t d) -> p t d", t=4)
    wp_sb = wall_sb[:, 9 * C:10 * C]
    bq_sb = ball_sb[:, 0:1]
    bp_sb = ball_sb[:, 1:2]

    # whole-batch activations + k/v, resident all kernel
    xT_sb = singles.tile([C, N], pdt)
    nc.sync.dma_start(out=xT_sb[:, 0:N // 2], in_=xT_d[:, 0:N // 2])
    nc.sync.dma_start(out=xT_sb[:, N // 2:N], in_=xT_d[:, N // 2:N])
    kT_sb = singles.tile([C, 7 * 128], pdt)
    nc.vector.memset(kT_sb[:, NKV:7 * 128], 0.0)
    vT_sb = singles.tile([C, NKV], pdt)
    v_sb = singles.tile([C, 7, HEADS, HD], pdt)

    state = {}
    pp_of = {}
    exp_of = {}

    def prep_tap(which, mc):
        """One kv-chunk of the fused conv-tap projection (4 matmuls)."""
        dst = kT_sb if which == 0 else vT_sb
        w_sb = wk_sb if which == 0 else wv_sb
        xview = xT_sb[:].rearrange("p (i a j c) -> p a c i j",
                                   i=28, a=2, j=28, c=2)
        ps_kv = ps_rot.tile([C, 2, 512], f32, tag="rot", name="ps_kv")
        ps_kv = ps_kv[:, 0, :]
        for t in range(SR * SR):
            di, dj = t // 2, t % 2
            rhs = xview[:, di, dj, 14 * mc:14 * mc + 14, :]
            nc.tensor.matmul(ps_kv[:, 0:392], lhsT=w_sb[:, t, :],
                             rhs=rhs, start=(t == 0), stop=(t == 3))
        nc.vector.tensor_copy(dst[:, 392 * mc:392 * (mc + 1)], ps_kv[:, 0:392])

    def prep_vtrans(j):
        m0, cnt = M_CHUNKS[j]
        ps_t = ps_rot.tile([C, 2, 512], pdt, tag="rot", name="ps_t")
        ps_t = ps_t[:, 0, :]
        nc.tensor.transpose(ps_t[0:cnt, 0:C], vT_sb[:, m0:m0 + cnt],
                            identb[:])
        nc.vector.tensor_copy(
            v_sb[0:cnt, j, :, :],
            ps_t[0:cnt, 0:C].rearrange("p (h d) -> p h d", h=HEADS, d=HD))

    def exp_load(nch, eng=None):
        """Prefetch the exp(rel) interleave for slot nch (4 DMAs: finer
        completion granularity lets early pieces start before the whole
        2.8MB table lands)."""
        eng = eng if eng is not None else nc.sync
        e_sb = epool.tile([C, 7, HEADS, NSL], pdt, tag="expTI", name="e_sb")
        exp_of[nch] = e_sb
        flat = e_sb[:].rearrange("p a h n -> p (a h n)")
        tot = 7 * HEADS * NSL
        qtr = tot // 4
        for i in range(4):
            eng.dma_start(out=flat[:, i * qtr:(i + 1) * qtr],
                          in_=expI_d[nch, :, i * qtr:(i + 1) * qtr])

    qT_sb = singles.tile([C, N], pdt)

    def prep_q(qc):
        """Whole-batch q projection, one slot's columns at fill time."""
        ps_q = ps_rot.tile([C, 2, 512], f32, tag="rot", name="ps_q")
        c0 = qc * NSL
        nc.tensor.matmul(ps_q[:, 0, 0:NSL], lhsT=wq_sb,
                         rhs=xT_sb[:, c0:c0 + NSL],
                         start=True, stop=True)
        nc.scalar.activation(qT_sb[:, c0:c0 + NSL],
                             ps_q[:, 0, 0:NSL], AF.Identity, bias=bq_sb)

    sco_of = {}

    def scores_pair(g, hp):
        """One head-pair of round g's score matmuls (2-way row packing).
        Pair 1 waits on the previous round's schraudolph ADD releasing its
        psum tile; emitting it one step later than pair 0 lets the ready
        attn@v/rowsum groups run during that wait instead of queuing
        behind it in the strict-FIFO Tensor queue."""
        if g >= NCH * 7:
            return
        nch, r = g // 7, g % 7
        ps_s = ps_rot.tile([C, 2, 512], f32, tag="rot", name="ps_s")
        sco_of.setdefault(g, [None, None])[hp] = ps_s
        for h in (2 * hp, 2 * hp + 1):
            nc.tensor.matmul(
                ps_s[0:128, h % 2, 0:NSL],
                lhsT=kT_sb[HD * h:HD * (h + 1), 128 * r:128 * (r + 1)],
                rhs=qT_sb[HD * h:HD * (h + 1), nch * NSL:(nch + 1) * NSL],
                start=True, stop=True,
                tile_position=(HD * h, 0))

    def scores_round(g):
        scores_pair(g, 0)
        scores_pair(g, 1)

    i16 = mybir.dt.int16

    def exp_mul(nch, r, hp):
        """exp(S+R) for chunk r, head pair hp: either exact (ScalarE exp of
        S*K/K then DVE/GpSimd multiply by the exp(R) table) or fused
        schraudolph (one DVE add of the int16 R-table onto S*K in PSUM,
        int16 result bits == bf16 exp)."""
        g = nch * 7 + r
        ps_s = sco_of[g][hp]
        if hp == 1:
            del sco_of[g]
        pslice = pp_of[nch][:, r, 2 * hp:2 * hp + 2, :]
        eslice = exp_of[nch][:, r, 2 * hp:2 * hp + 2, :]
        if (r, hp) in SCH_PIECES:
            nc.vector.tensor_add(pslice.bitcast(i16), ps_s[:, :, 0:NSL],
                                 eslice.bitcast(i16))
            return
        pt_sb = ptpool.tile([C, 2 * NSL], pdt, tag="pt")
        nc.scalar.activation(pt_sb[:], ps_s[:, :, 0:NSL], AF.Exp,
                             scale=1.0 / K_SCH)
        eng = nc.gpsimd if (r, hp) in GPS_MULS else nc.vector
        off = (r * HEADS + 2 * hp) * NSL
        ppf = pp_of[nch][:].rearrange("p a h n -> p (a h n)")
        ef = exp_of[nch][:].rearrange("p a h n -> p (a h n)")
        eng.tensor_mul(ppf[:, off:off + 2 * NSL], pt_sb[:],
                       ef[:, off:off + 2 * NSL])

    def attnv4(nch, r, part=None):
        """attn@v + rowsums for kv chunk r: all 4 heads column-packed.
        part='av' emits only attn@v, 'rs' only rowsums (lets the two groups
        land on different pipeline steps). The last slot accumulates in a
        retired scores-rotation tile so its rounds can start before the
        previous slot's extract frees ov/z."""
        s = state[nch]
        m0, cnt = M_CHUNKS[r]
        if r == 0 and "ov" not in s:
            if nch == NCH - 1:
                ovz = ps_rot.tile([C, 2, 512], f32, tag="rot", name="ps_ovz")
                s["ov"] = ovz[:, 0, :]
                s["z"] = ovz[:, 1, :]
            else:
                s["ov"] = ps_out.tile([C, 512], f32, tag="out", name="ps_ov")
                s["z"] = ps_rs.tile([C, 512], f32, tag="rs", name="ps_z")
        ps_ov, ps_z = s["ov"], s["z"]
        pp = pp_of[nch]
        if part in (None, "av"):
            for h in range(HEADS):
                nc.tensor.matmul(
                    ps_ov[HD * h:HD * (h + 1), 0:NSL],
                    lhsT=v_sb[0:cnt, r, h, :],
                    rhs=pp[0:cnt, r, h, :],
                    start=(r == 0), stop=(r == len(M_CHUNKS) - 1),
                    tile_position=(0, HD * h), skip_group_check=True)
        if part in (None, "rs"):
            for h in range(HEADS):
                nc.tensor.matmul(
                    ps_z[HD * h:HD * (h + 1), 0:NSL],
                    lhsT=ones_sb[0:cnt, :],
                    rhs=pp[0:cnt, r, h, :],
                    start=(r == 0), stop=(r == len(M_CHUNKS) - 1),
                    tile_position=(0, HD * h), skip_group_check=True)

    def extract(nch):
        """Normalize straight out of PSUM: recip(rowsums), multiply."""
        s = state[nch]
        ps_ov = s.pop("ov")
        ps_z = s.pop("z")
        rb_sb = opool.tile([C, NSL], f32, tag="rb")
        nc.vector.reciprocal_approx_fast(rb_sb[:], ps_z[0:C, 0:NSL])
        outT_sb = opool.tile([C, NSL], pdt, tag="outT")
        s["outT"] = outT_sb
        nc.vector.tensor_mul(outT_sb[:], ps_ov[0:C, 0:NSL], rb_sb[:])

    def proj_tail(nch):
        """Final projection in transposed layout; host untransposes."""
        s = state[nch]
        ps_ft = ps_rot.tile([C, 2, 512], f32, tag="rot", name="ps_ft")
        ps_ft = ps_ft[:, 0, :]
        nc.tensor.matmul(ps_ft[0:C, 0:NSL], lhsT=wp_sb,
                         rhs=s.pop("outT")[:], start=True, stop=True)
        fin_sb = opool.tile([C, NSL], f32, tag="fin")
        # bias-add on ScalarE: DVE is loaded with schraudolph/mul work
        nc.scalar.activation(fin_sb[:], ps_ft[0:C, 0:NSL],
                             AF.Identity, bias=bp_sb)
        nc.sync.dma_start(out=out_d[:, nch * NSL:(nch + 1) * NSL],
                          in_=fin_sb[:])
        state.pop(nch)
        pp_of.pop(nch, None)
        exp_of.pop(nch, None)

    # ---- fill: k/v + all of q once, first exp tables ----
    # exp tables ride the sync ring: the scalar sequencer must stay free to
    # issue the q-bias IDENTITYs (ring backpressure from 2.8MB table DMAs
    # otherwise stalls everything queued behind them on that engine).
    exp_load(0, nc.sync)
    # scores_round(0) needs only kT chunk 0 (kv cols 0:128 -- inside the
    # first k-tap half) and the first q slot; everything else (second
    # k-tap half, v-taps, remaining q) comes off the critical path.
    prep_tap(0, 0)
    prep_q(0)
    scores_round(0)
    prep_tap(0, 1)
    for mc in range(2):
        prep_tap(1, mc)
    for qc in range(1, NCH):
        prep_q(qc)
    for j in range(7):
        prep_vtrans(j)
    exp_load(1, nc.sync)
    # ---- steady loop over n-chunk slots ----
    for nch in range(NCH):
        pp_of[nch] = ppool.tile([C, 7, HEADS, NSL], pdt, tag="pp",
                                name="pp_sb")
        state.setdefault(nch, {})
        for step in range(14):
            r, hp = step // 2, step % 2
            scores_pair(nch * 7 + r + 1, hp)
            exp_mul(nch, r, hp)
            if nch >= 1:
                if hp == 0 and step <= 12:
                    attnv4(nch - 1, step // 2)
                elif step == 13:
                    extract(nch - 1)
            if nch == NCH - 1 and step >= 12:
                attnv4(nch, step - 12)
            if nch >= 2 and step == 0:
                proj_tail(nch - 2)
            if step == 1 and nch + 2 < NCH:
                exp_load(nch + 2)
    # drain
    proj_tail(NCH - 2)
    for r in range(2, 7):
        attnv4(NCH - 1, r)
    extract(NCH - 1)
    proj_tail(NCH - 1)


def _get_compiled():
    global _COMPILED
    if _COMPILED is None:
        _COMPILED = _build()
    return _COMPILED


def make_in_map(prep, j):
    return {
        "xT": np.ascontiguousarray(prep["xT"][j]),
        "expI": prep["expI"].reshape(NCH, C, 7 * HEADS * NSL),
        "Wall": prep["Wall"], "ball": prep["ball"],
    }


def kernel(x, relative_pos, Wq, bq, Wk, bk, Wv, bv, conv_w, conv_b,
           bn_gamma, bn_beta, bn_mean, bn_var, Wp, bp, H=56, W=56,
           _trace=False):
    from concourse.bass_utils import run_bass_kernel_spmd

    prep = _host_prep(x, relative_pos, Wq, bq, Wk, bk, Wv, bv, conv_w,
                      conv_b, bn_gamma, bn_beta, bn_mean, bn_var, Wp, bp)
    nc = _get_compiled()

    in_maps = [make_in_map(prep, j) for j in range(NCORES)]

    res = run_bass_kernel_spmd(nc, in_maps, core_ids=list(range(NCORES)),
                               trace=_trace)

    out = np.empty((B, N, C), np.float32)
    for j in range(NCORES):
        out[j] = res.results[j]["out"].T
    if _trace:
        kernel._last_result = res
    return out



# revision 35
# speedup vs baseline: 1.0108x; 1.0021x over previous
"""Trainium2 Bass kernel for PVT-style spatial-reduction attention.

Model (see reference):
  q = (x @ Wq + bq) * hd^-0.5                       (B, N, C) -> heads of 32
  x_ = BN(DWConv2x2s2(x)) ; k = x_ @ Wk + bk ; v = x_ @ Wv + bv
  attn = softmax(q k^T + rel_pos) ; out = (attn @ v) @ Wp + bp

Shapes: B=8, N=3136 (56x56), C=128, heads=4, hd=32, Nkv=784 (28x28).

Distribution: data-parallel over batch -- core j handles batch j fully
(B == n_cores == 8).  k/v/conv-taps are computed once per core (vs 8x
redundantly under query-sharding), cutting TensorE work by a third; the
exp(rel_pos) table is streamed per n-chunk from HBM instead.

Device layout strategy: features-on-partitions everywhere (C == 128).
  - host passes xT (B, C, N) in bf16; all projections are lhsT=weight
    matmuls.
  - conv+BN+k/v projection fused into 4 "tap" weight matrices (host
    precomputed); k-bias dropped (softmax-invariant), v-bias folded into
    the final bias.
  - scores computed transposed: S^T[m, n] per (nch, h), scaled by
    K = 128/ln2 on the weight side.  Softmax numerator per (r, hp) piece:
      hp==0 pieces: fused Schraudolph fast-exp -- ONE DVE tensor_add of a
        host-built int16 table round(K*R + 16256 - C) onto the f32 psum
        scores; the int16 sum IS the bf16 bit pattern of exp(S+R)
        (max ~4% per-weight error, geometric mean calibrated to 1 via C;
        net output error ~1.3e-3).  Also releases the scores psum tile
        fast, which the next round's pair-2 matmuls wait on.
      hp==1 pieces: exact ScalarE exp(S) (scale=1/K) then multiply by the
        bf16 exp(R) table on DVE or GpSimd (dense flat APs -> 2x DVE mode).
  - per (n-chunk, kv-chunk): attn@v runs all 4 heads 4-way column-packed
    (32-wide) into ONE psum bank; row sums via 4 column-packed all-ones
    matmuls (32-replicated) into a second bank, which doubles as the
    softmax-denominator broadcast: extract = reciprocal + multiply only.
  - the last slot accumulates attn@v in a retired scores-rotation psum
    tile so its chunks start inside the steady loop (shorter drain).
  - one-DMA weight load; biases on the sync ring; exp tables on the sync
    ring (scalar-ring backpressure stalled the q-bias IDENTITYs).
  - software pipeline per n-chunk slot (14 half-round steps):
      all steps:   scores+exp+mul of slot (kv chunk step//2, pair step%2)
      even steps:  attn@v + rowsums of slot-1, one kv chunk per step
      step 13:     extract (normalize) of slot-1
      step 0:      projection tail of slot-2
      step 1:      q projection of slot+1; expTI prefetch of slot+2
  - final output is produced transposed (C, N) per core; the host
    untransposes while assembling the full (B, N, C) result.
"""

import os
import sys

import numpy as np

if "/opt/trn_rl_repo" not in sys.path:
    sys.path.insert(0, "/opt/trn_rl_repo")

B = 8
N = 3136
C = 128
HEADS = 4
HD = 32
SR = 2
H = W = 56
NKV = 784  # 28*28
NCORES = 8
NSL = 448          # query rows per n-chunk slot (448 <= 512 psum bank)
NCH = N // NSL     # 7 slots per core
BN_EPS = 1e-5
SCALE = HD ** -0.5

# m (kv index) chunking: 784 = 6*128 + 16
M_CHUNKS = [(j * 128, min(128, NKV - j * 128)) for j in range((NKV + 127) // 128)]

PROB_BF16 = os.environ.get("KERNEL_PROB_BF16", "1") == "1"


def _parse_pieces(env, default):
    v = os.environ.get(env)
    if not v:
        return default
    return {tuple(int(x) for x in p.split(",")) for p in v.split(";") if p}


# (r, hp) half-rounds computed via fused Schraudolph fast-exp on DVE
# (exp(S+R) ~= bf16-bits of int16(K*(S+R) + B - C)); the rest use exact
# ScalarE exp + table multiply.  All hp==0 pieces use the fast path so the
# scores psum-tile rotation is released by a single quick DVE op (the
# next scores round's pair-2 waits on it); measured best balance:
# Scalar ~60us, DVE ~80us, GpSimd ~41us vs TensorE ~100us (the pacer).
SCH_PIECES = _parse_pieces(
    "KERNEL_SCH", {(r, 0) for r in range(7)})
# exact pieces whose exp(R) multiply runs on GpSimd instead of DVE.
# NOTE: 5-element GPS sets produced hardware-only corruption (see log);
# keep <= 4 pieces here.
GPS_MULS = _parse_pieces("KERNEL_GPS", {(0, 1), (2, 1), (4, 1)}) \
    - SCH_PIECES
K_SCH = 2.0 ** 7 / np.log(2.0)          # 184.665 (exp -> bf16-bit scale)
B_SCH = 127.0 * 128.0                   # bf16 exponent bias in bit units
C_SCH = float(os.environ.get("KERNEL_SCH_C", "7.333"))
if not PROB_BF16:
    SCH_PIECES = set()  # int16 bit-pun needs the bf16 table container

_COMPILED = None  # cached nc across kernel() calls
_PREP_CACHE = {}  # host-prep results cached by input id


def _host_prep(x, relative_pos, Wq, bq, Wk, bk, Wv, bv, conv_w, conv_b,
               bn_gamma, bn_beta, bn_mean, bn_var, Wp, bp):
    """Fuse conv/BN into tap weights; fold biases; transpose activations."""
    import ml_dtypes
    f32 = np.float32
    bf16 = ml_dtypes.bfloat16
    wdt = bf16 if PROB_BF16 else f32
    x = np.asarray(x, f32)
    # xT: (B, C, N)
    xT = np.ascontiguousarray(x.transpose(0, 2, 1).astype(wdt))

    inv = (np.asarray(bn_gamma, f32)
           / np.sqrt(np.asarray(bn_var, f32) + BN_EPS))          # [c]
    wp_taps = np.asarray(conv_w, f32).reshape(C, SR * SR) * inv[:, None]  # [c,4]
    beta0 = (np.asarray(conv_b, f32) * inv
             + np.asarray(bn_beta, f32)
             - np.asarray(bn_mean, f32) * inv)                    # [c]

    Wk = np.asarray(Wk, f32)
    Wv = np.asarray(Wv, f32)
    # Wk_tap[t, c, c'] = wp_taps[c, t] * Wk[c, c']
    Wk_tap = np.ascontiguousarray(
        (wp_taps.T[:, :, None] * Wk[None, :, :]).astype(wdt))     # (4, C, C)
    Wv_tap = np.ascontiguousarray(
        (wp_taps.T[:, :, None] * Wv[None, :, :]).astype(wdt))

    # v bias (uniform over kv positions -> exact fold into final bias)
    beta_v = beta0 @ Wv + np.asarray(bv, f32)                     # [c']
    bp_col = (np.asarray(bp, f32) + beta_v @ np.asarray(Wp, f32)).reshape(C, 1)

    # scores are computed as S*K_SCH on device (Wq pre-scaled); the exact
    # pieces undo it inside the activation (scale=1/K_SCH), the schraudolph
    # pieces consume it directly.
    Wq_s = np.ascontiguousarray(
        (np.asarray(Wq, f32) * (SCALE * K_SCH)).astype(wdt))
    bq_col = (np.asarray(bq, f32) * (SCALE * K_SCH)).reshape(C, 1)

    # rel^T table interleaved per n-chunk: (NCH, C, 7, HEADS, NSL).
    # Exact pieces (r, hp) store exp(R) in bf16; schraudolph pieces store
    # int16(K*R + B - C) bit-punned into the same bf16 container.
    rel = np.asarray(relative_pos, f32)                  # (4, N, NKV)
    rT = rel.transpose(0, 2, 1)                          # (4, NKV, N) f32
    expI = np.zeros((NCH, C, 7, HEADS, NSL), wdt)
    for j, (m0, cnt) in enumerate(M_CHUNKS):
        # (4, cnt, NCH, NSL) -> (NCH, cnt, h, NSL)
        blk = rT[:, m0:m0 + cnt, :].reshape(HEADS, cnt, NCH, NSL)
        blk = blk.transpose(2, 1, 0, 3)                  # (NCH, cnt, h, NSL)
        for hp in range(2):
            part = blk[:, :, 2 * hp:2 * hp + 2, :]
            if (j, hp) in SCH_PIECES:
                t = np.round(part * K_SCH + (B_SCH - C_SCH))
                dst = t.astype(np.int16).view(wdt)
            else:
                dst = np.exp(part).astype(wdt)
            expI[:, 0:cnt, j, 2 * hp:2 * hp + 2, :] = dst
    expI = np.ascontiguousarray(expI)

    # concat all bf16 weights into one [C, 10C] tensor (one DMA):
    # wq | wk taps (4C, interleaved t-major per c) | wv taps | wp
    Wp_c = np.asarray(Wp, f32).astype(wdt)
    wall = np.concatenate(
        [Wq_s,
         Wk_tap.transpose(1, 0, 2).reshape(C, 4 * C),
         Wv_tap.transpose(1, 0, 2).reshape(C, 4 * C),
         Wp_c], axis=1)
    ball = np.concatenate([bq_col, bp_col], axis=1)  # [C, 2] f32

    return dict(xT=xT, Wall=np.ascontiguousarray(wall),
                ball=np.ascontiguousarray(ball), expI=expI)


def _build():
    """Build + compile the SPMD bass program (same NEFF for all 8 cores)."""
    import concourse.bass as bass
    import concourse.tile as tile
    from concourse import bacc, mybir
    from concourse.masks import make_identity

    f32 = mybir.dt.float32
    f32r = mybir.dt.float32r
    pdt = mybir.dt.bfloat16 if PROB_BF16 else f32

    nc = bacc.Bacc("TRN2", target_bir_lowering=False, debug=False,
                   num_devices=NCORES)

    # ---- DRAM I/O ----
    xT_d = nc.dram_tensor("xT", [C, N], pdt, kind="ExternalInput").ap()
    expI_d = nc.dram_tensor("expI", [NCH, C, 7 * HEADS * NSL], pdt,
                            kind="ExternalInput").ap()
    Wall_d = nc.dram_tensor("Wall", [C, 10 * C], pdt,
                            kind="ExternalInput").ap()
    ball_d = nc.dram_tensor("ball", [C, 2], f32, kind="ExternalInput").ap()
    out_d = nc.dram_tensor("out", [C, N], f32, kind="ExternalOutput").ap()

    with tile.TileContext(nc) as tc:
        from contextlib import ExitStack
        with ExitStack() as ctx:
            _emit(ctx, tc, nc, bass, mybir, make_identity, f32, f32r, pdt,
                  xT_d, expI_d, Wall_d, ball_d, out_d)

    nc.compile()
    return nc


def _emit(ctx, tc, nc, bass, mybir, make_identity, f32, f32r, pdt,
          xT_d, expI_d, Wall_d, ball_d, out_d):
    AF = mybir.ActivationFunctionType

    singles = ctx.enter_context(tc.tile_pool(name="singles", bufs=1))
    ppool = ctx.enter_context(tc.tile_pool(name="ppool", bufs=3))
    epool = ctx.enter_context(tc.tile_pool(name="epool", bufs=3))
    opool = ctx.enter_context(tc.tile_pool(name="opool", bufs=3))
    qpool = ctx.enter_context(tc.tile_pool(name="qpool", bufs=3))
    ptpool = ctx.enter_context(tc.tile_pool(name="ptpool", bufs=6))
    # PSUM: rot 3x2 + out 1 + rs 1 = 8 banks
    ps_rot = ctx.enter_context(tc.tile_pool(name="ps_rot", bufs=3,
                                            space="PSUM"))
    ps_out = ctx.enter_context(tc.tile_pool(name="ps_out", bufs=1,
                                            space="PSUM"))
    ps_rs = ctx.enter_context(tc.tile_pool(name="ps_rs", bufs=1,
                                           space="PSUM"))

    # ---- constants ----
    identb = singles.tile([C, C], pdt)
    make_identity(nc, identb[:])
    ones_sb = singles.tile([C, HD], pdt)
    nc.vector.memset(ones_sb[:], 1.0)

    # all weights in ONE DMA (the ~700ns/DMA sequencer issue cost was
    # serializing the fill); biases in a second small one.
    wall_sb = singles.tile([C, 10 * C], pdt)
    nc.scalar.dma_start(out=wall_sb[:], in_=Wall_d)
    # biases ride the (quiet) sync ring so their completion isn't lumped
    # behind the exp-table loads on the scalar ring
    ball_sb = singles.tile([C, 2], f32)
    nc.sync.dma_start(out=ball_sb[:], in_=ball_d)
    wq_sb = wall_sb[:, 0:C]
    wk_sb = wall_sb[:, C:5 * C].rearrange("p (t d) -> p t d", t=4)
    wv_sb = wall_sb[:, 5 * C:9 * C].rearrange("p (t d) -> p t d", t=4)
    wp_sb = wall_sb[:, 9 * C:10 * C]
    bq_sb = ball_sb[:, 0:1]
    bp_sb = ball_sb[:, 1:2]

    # whole-batch activations + k/v, resident all kernel
    xT_sb = singles.tile([C, N], pdt)
    nc.sync.dma_start(out=xT_sb[:, 0:N // 2], in_=xT_d[:, 0:N // 2])
    nc.sync.dma_start(out=xT_sb[:, N // 2:N], in_=xT_d[:, N // 2:N])
    kT_sb = singles.tile([C, 7 * 128], pdt)
    nc.vector.memset(kT_sb[:, NKV:7 * 128], 0.0)
    vT_sb = singles.tile([C, NKV], pdt)
    v_sb = singles.tile([C, 7, HEADS, HD], pdt)

    state = {}
    pp_of = {}
    exp_of = {}

    def prep_tap(which, mc):
        """One kv-chunk of the fused conv-tap projection (4 matmuls)."""
        dst = kT_sb if which == 0 else vT_sb
        w_sb = wk_sb if which == 0 else wv_sb
        xview = xT_sb[:].rearrange("p (i a j c) -> p a c i j",
                                   i=28, a=2, j=28, c=2)
        ps_kv = ps_rot.tile([C, 2, 512], f32, tag="rot", name="ps_kv")
        ps_kv = ps_kv[:, 0, :]
        for t in range(SR * SR):
            di, dj = t // 2, t % 2
            rhs = xview[:, di, dj, 14 * mc:14 * mc + 14, :]
            nc.tensor.matmul(ps_kv[:, 0:392], lhsT=w_sb[:, t, :],
                             rhs=rhs, start=(t == 0), stop=(t == 3))
        nc.vector.tensor_copy(dst[:, 392 * mc:392 * (mc + 1)], ps_kv[:, 0:392])

    def prep_vtrans(j):
        m0, cnt = M_CHUNKS[j]
        ps_t = ps_rot.tile([C, 2, 512], pdt, tag="rot", name="ps_t")
        ps_t = ps_t[:, 0, :]
        nc.tensor.transpose(ps_t[0:cnt, 0:C], vT_sb[:, m0:m0 + cnt],
                            identb[:])
        nc.vector.tensor_copy(
            v_sb[0:cnt, j, :, :],
            ps_t[0:cnt, 0:C].rearrange("p (h d) -> p h d", h=HEADS, d=HD))

    def exp_load(nch, eng=None):
        """Prefetch the exp(rel) interleave for slot nch (4 DMAs: finer
        completion granularity lets early pieces start before the whole
        2.8MB table lands)."""
        eng = eng if eng is not None else nc.sync
        e_sb = epool.tile([C, 7, HEADS, NSL], pdt, tag="expTI", name="e_sb")
        exp_of[nch] = e_sb
        flat = e_sb[:].rearrange("p a h n -> p (a h n)")
        tot = 7 * HEADS * NSL
        qtr = tot // 4
        for i in range(4):
            eng.dma_start(out=flat[:, i * qtr:(i + 1) * qtr],
                          in_=expI_d[nch, :, i * qtr:(i + 1) * qtr])

    qT_sb = singles.tile([C, N], pdt)

    def prep_q(qc):
        """Whole-batch q projection, one slot's columns at fill time."""
        ps_q = ps_rot.tile([C, 2, 512], f32, tag="rot", name="ps_q")
        c0 = qc * NSL
        nc.tensor.matmul(ps_q[:, 0, 0:NSL], lhsT=wq_sb,
                         rhs=xT_sb[:, c0:c0 + NSL],
                         start=True, stop=True)
        nc.scalar.activation(qT_sb[:, c0:c0 + NSL],
                             ps_q[:, 0, 0:NSL], AF.Identity, bias=bq_sb)

    sco_of = {}

    def scores_pair(g, hp):
        """One head-pair of round g's score matmuls (2-way row packing).
        Pair 1 waits on the previous round's schraudolph ADD releasing its
        psum tile; emitting it one step later than pair 0 lets the ready
        attn@v/rowsum groups run during that wait instead of queuing
        behind it in the strict-FIFO Tensor queue."""
        if g >= NCH * 7:
            return
        nch, r = g // 7, g % 7
        ps_s = ps_rot.tile([C, 2, 512], f32, tag="rot", name="ps_s")
        sco_of.setdefault(g, [None, None])[hp] = ps_s
        for h in (2 * hp, 2 * hp + 1):
            nc.tensor.matmul(
                ps_s[0:128, h % 2, 0:NSL],
                lhsT=kT_sb[HD * h:HD * (h + 1), 128 * r:128 * (r + 1)],
                rhs=qT_sb[HD * h:HD * (h + 1), nch * NSL:(nch + 1) * NSL],
                start=True, stop=True,
                tile_position=(HD * h, 0))

    def scores_round(g):
        scores_pair(g, 0)
        scores_pair(g, 1)

    i16 = mybir.dt.int16

    def exp_mul(nch, r, hp):
        """exp(S+R) for chunk r, head pair hp: either exact (ScalarE exp of
        S*K/K then DVE/GpSimd multiply by the exp(R) table) or fused
        schraudolph (one DVE add of the int16 R-table onto S*K in PSUM,
        int16 result bits == bf16 exp)."""
        g = nch * 7 + r
        ps_s = sco_of[g][hp]
        if hp == 1:
            del sco_of[g]
        pslice = pp_of[nch][:, r, 2 * hp:2 * hp + 2, :]
        eslice = exp_of[nch][:, r, 2 * hp:2 * hp + 2, :]
        if (r, hp) in SCH_PIECES:
            nc.vector.tensor_add(pslice.bitcast(i16), ps_s[:, :, 0:NSL],
                                 eslice.bitcast(i16))
            return
        pt_sb = ptpool.tile([C, 2 * NSL], pdt, tag="pt")
        nc.scalar.activation(pt_sb[:], ps_s[:, :, 0:NSL], AF.Exp,
                             scale=1.0 / K_SCH)
        eng = nc.gpsimd if (r, hp) in GPS_MULS else nc.vector
        off = (r * HEADS + 2 * hp) * NSL
        ppf = pp_of[nch][:].rearrange("p a h n -> p (a h n)")
        ef = exp_of[nch][:].rearrange("p a h n -> p (a h n)")
        eng.tensor_mul(ppf[:, off:off + 2 * NSL], pt_sb[:],
                       ef[:, off:off + 2 * NSL])

    def attnv4(nch, r, part=None):
        """attn@v + rowsums for kv chunk r: all 4 heads column-packed.
        part='av' emits only attn@v, 'rs' only rowsums (lets the two groups
        land on different pipeline steps). The last slot accumulates in a
        retired scores-rotation tile so its rounds can start before the
        previous slot's extract frees ov/z."""
        s = state[nch]
        m0, cnt = M_CHUNKS[r]
        if r == 0 and "ov" not in s:
            if nch == NCH - 1:
                ovz = ps_rot.tile([C, 2, 512], f32, tag="rot", name="ps_ovz")
                s["ov"] = ovz[:, 0, :]
                s["z"] = ovz[:, 1, :]
            else:
                s["ov"] = ps_out.tile([C, 512], f32, tag="out", name="ps_ov")
                s["z"] = ps_rs.tile([C, 512], f32, tag="rs", name="ps_z")
        ps_ov, ps_z = s["ov"], s["z"]
        pp = pp_of[nch]
        if part in (None, "av"):
            for h in range(HEADS):
                nc.tensor.matmul(
                    ps_ov[HD * h:HD * (h + 1), 0:NSL],
                    lhsT=v_sb[0:cnt, r, h, :],
                    rhs=pp[0:cnt, r, h, :],
                    start=(r == 0), stop=(r == len(M_CHUNKS) - 1),
                    tile_position=(0, HD * h), skip_group_check=True)
        if part in (None, "rs"):
            for h in range(HEADS):
                nc.tensor.matmul(
                    ps_z[HD * h:HD * (h + 1), 0:NSL],
                    lhsT=ones_sb[0:cnt, :],
                    rhs=pp[0:cnt, r, h, :],
                    start=(r == 0), stop=(r == len(M_CHUNKS) - 1),
                    tile_position=(0, HD * h), skip_group_check=True)

    def extract(nch):
        """Normalize straight out of PSUM: recip(rowsums), multiply."""
        s = state[nch]
        ps_ov = s.pop("ov")
        ps_z = s.pop("z")
        rb_sb = opool.tile([C, NSL], f32, tag="rb")
        nc.vector.reciprocal_approx_fast(rb_sb[:], ps_z[0:C, 0:NSL])
        outT_sb = opool.tile([C, NSL], pdt, tag="outT")
        s["outT"] = outT_sb
        nc.vector.tensor_mul(outT_sb[:], ps_ov[0:C, 0:NSL], rb_sb[:])

    def proj_tail(nch):
        """Final projection in transposed layout; host untransposes."""
        s = state[nch]
        ps_ft = ps_rot.tile([C, 2, 512], f32, tag="rot", name="ps_ft")
        ps_ft = ps_ft[:, 0, :]
        nc.tensor.matmul(ps_ft[0:C, 0:NSL], lhsT=wp_sb,
                         rhs=s.pop("outT")[:], start=True, stop=True)
        fin_sb = opool.tile([C, NSL], f32, tag="fin")
        # bias-add on ScalarE: DVE is loaded with schraudolph/mul work
        nc.scalar.activation(fin_sb[:], ps_ft[0:C, 0:NSL],
                             AF.Identity, bias=bp_sb)
        nc.sync.dma_start(out=out_d[:, nch * NSL:(nch + 1) * NSL],
                          in_=fin_sb[:])
        state.pop(nch)
        pp_of.pop(nch, None)
        exp_of.pop(nch, None)

    # ---- fill: k/v + all of q once, first exp tables ----
    # exp tables ride the sync ring: the scalar sequencer must stay free to
    # issue the q-bias IDENTITYs (ring backpressure from 2.8MB table DMAs
    # otherwise stalls everything queued behind them on that engine).
    exp_load(0, nc.sync)
    # scores_round(0) needs only kT chunk 0 (kv cols 0:128 -- inside the
    # first k-tap half) and the first q slot; everything else (second
    # k-tap half, v-taps, remaining q) comes off the critical path.
    prep_tap(0, 0)
    prep_q(0)
    scores_round(0)
    prep_tap(0, 1)
    for mc in range(2):
        prep_tap(1, mc)
    for qc in range(1, NCH):
        prep_q(qc)
    for j in range(7):
        prep_vtrans(j)
    exp_load(1, nc.sync)
    # ---- steady loop over n-chunk slots ----
    for nch in range(NCH):
        pp_of[nch] = ppool.tile([C, 7, HEADS, NSL], pdt, tag="pp",
                                name="pp_sb")
        state.setdefault(nch, {})
        for step in range(14):
            r, hp = step // 2, step % 2
            scores_pair(nch * 7 + r + 1, hp)
            exp_mul(nch, r, hp)
            if nch >= 1:
                if hp == 0 and step <= 12:
                    attnv4(nch - 1, step // 2)
                elif step == 13:
                    extract(nch - 1)
            if nch == NCH - 1 and step >= 12:
                # no more score rounds by now -- pack the last slot's attn@v
                # chunks into the PE-idle window while the final pieces run
                attnv4(nch, 2 * (step - 12))
                attnv4(nch, 2 * (step - 12) + 1)
            if nch >= 2 and step == 0:
                proj_tail(nch - 2)
            if step == 1 and nch + 2 < NCH:
                exp_load(nch + 2)
    # drain
    proj_tail(NCH - 2)
    for r in range(4, 7):
        attnv4(NCH - 1, r)
    extract(NCH - 1)
    proj_tail(NCH - 1)


def _get_compiled():
    global _COMPILED
    if _COMPILED is None:
        _COMPILED = _build()
    return _COMPILED


def make_in_map(prep, j):
    return {
        "xT": np.ascontiguousarray(prep["xT"][j]),
        "expI": prep["expI"].reshape(NCH, C, 7 * HEADS * NSL),
        "Wall": prep["Wall"], "ball": prep["ball"],
    }


def kernel(x, relative_pos, Wq, bq, Wk, bk, Wv, bv, conv_w, conv_b,
           bn_gamma, bn_beta, bn_mean, bn_var, Wp, bp, H=56, W=56,
           _trace=False):
    from concourse.bass_utils import run_bass_kernel_spmd

    prep = _host_prep(x, relative_pos, Wq, bq, Wk, bk, Wv, bv, conv_w,
                      conv_b, bn_gamma, bn_beta, bn_mean, bn_var, Wp, bp)
    nc = _get_compiled()

    in_maps = [make_in_map(prep, j) for j in range(NCORES)]

    res = run_bass_kernel_spmd(nc, in_maps, core_ids=list(range(NCORES)),
                               trace=_trace)

    out = np.empty((B, N, C), np.float32)
    for j in range(NCORES):
        out[j] = res.results[j]["out"].T
    if _trace:
        kernel._last_result = res
    return out



# revision 36
# speedup vs baseline: 1.0646x; 1.0532x over previous
"""Trainium2 Bass kernel for PVT-style spatial-reduction attention.

Model (see reference):
  q = (x @ Wq + bq) * hd^-0.5                       (B, N, C) -> heads of 32
  x_ = BN(DWConv2x2s2(x)) ; k = x_ @ Wk + bk ; v = x_ @ Wv + bv
  attn = softmax(q k^T + rel_pos) ; out = (attn @ v) @ Wp + bp

Shapes: B=8, N=3136 (56x56), C=128, heads=4, hd=32, Nkv=784 (28x28).

Distribution: data-parallel over batch -- core j handles batch j fully
(B == n_cores == 8).  k/v/conv-taps are computed once per core (vs 8x
redundantly under query-sharding), cutting TensorE work by a third; the
exp(rel_pos) table is streamed per n-chunk from HBM instead.

Device layout strategy: features-on-partitions everywhere (C == 128).
  - host passes xT (B, C, N) in bf16; all projections are lhsT=weight
    matmuls.
  - conv+BN+k/v projection fused into 4 "tap" weight matrices (host
    precomputed); k-bias dropped (softmax-invariant), v-bias folded into
    the final bias.
  - scores computed transposed: S^T[m, n] per (nch, h), scaled by
    K = 128/ln2 on the weight side.  Softmax numerator per (r, hp) piece:
      hp==0 pieces: fused Schraudolph fast-exp -- ONE DVE tensor_add of a
        host-built int16 table round(K*R + 16256 - C) onto the f32 psum
        scores; the int16 sum IS the bf16 bit pattern of exp(S+R)
        (max ~4% per-weight error, geometric mean calibrated to 1 via C;
        net output error ~1.3e-3).  Also releases the scores psum tile
        fast, which the next round's pair-2 matmuls wait on.
      hp==1 pieces: exact ScalarE exp(S) (scale=1/K) then multiply by the
        bf16 exp(R) table on DVE or GpSimd (dense flat APs -> 2x DVE mode).
  - per (n-chunk, kv-chunk): attn@v runs all 4 heads 4-way column-packed
    (32-wide) into ONE psum bank; row sums via 4 column-packed all-ones
    matmuls (32-replicated) into a second bank, which doubles as the
    softmax-denominator broadcast: extract = reciprocal + multiply only.
  - the last slot accumulates attn@v in a retired scores-rotation psum
    tile so its chunks start inside the steady loop (shorter drain).
  - one-DMA weight load; biases on the sync ring; exp tables on the sync
    ring (scalar-ring backpressure stalled the q-bias IDENTITYs).
  - software pipeline per n-chunk slot (14 half-round steps):
      all steps:   scores+exp+mul of slot (kv chunk step//2, pair step%2)
      even steps:  attn@v + rowsums of slot-1, one kv chunk per step
      step 13:     extract (normalize) of slot-1
      step 0:      projection tail of slot-2
      step 1:      q projection of slot+1; expTI prefetch of slot+2
  - final output is produced transposed (C, N) per core; the host
    untransposes while assembling the full (B, N, C) result.
"""

import os
import sys

import numpy as np

if "/opt/trn_rl_repo" not in sys.path:
    sys.path.insert(0, "/opt/trn_rl_repo")

B = 8
N = 3136
C = 128
HEADS = 4
HD = 32
SR = 2
H = W = 56
NKV = 784  # 28*28
NCORES = 8
NSL = 448          # query rows per n-chunk slot (448 <= 512 psum bank)
NCH = N // NSL     # 7 slots per core
BN_EPS = 1e-5
SCALE = HD ** -0.5

# m (kv index) chunking: 784 = 6*128 + 16
M_CHUNKS = [(j * 128, min(128, NKV - j * 128)) for j in range((NKV + 127) // 128)]

PROB_BF16 = os.environ.get("KERNEL_PROB_BF16", "1") == "1"


def _parse_pieces(env, default):
    v = os.environ.get(env)
    if not v:
        return default
    return {tuple(int(x) for x in p.split(",")) for p in v.split(";") if p}


# (r, hp) half-rounds computed via fused Schraudolph fast-exp on DVE
# (exp(S+R) ~= bf16-bits of int16(K*(S+R) + B - C)); the rest use exact
# ScalarE exp + table multiply.  All hp==0 pieces use the fast path so the
# scores psum-tile rotation is released by a single quick DVE op (the
# next scores round's pair-2 waits on it); measured best balance:
# Scalar ~60us, DVE ~80us, GpSimd ~41us vs TensorE ~100us (the pacer).
SCH_PIECES = _parse_pieces(
    "KERNEL_SCH", {(r, 0) for r in range(7)})
# exact pieces whose exp(R) multiply runs on GpSimd instead of DVE.
# NOTE: 5-element GPS sets produced hardware-only corruption (see log);
# keep <= 4 pieces here.
GPS_MULS = _parse_pieces("KERNEL_GPS",
                         {(0, 1), (2, 1), (4, 1), (6, 1)}) \
    - SCH_PIECES
K_SCH = 2.0 ** 7 / np.log(2.0)          # 184.665 (exp -> bf16-bit scale)
B_SCH = 127.0 * 128.0                   # bf16 exponent bias in bit units
C_SCH = float(os.environ.get("KERNEL_SCH_C", "7.333"))
if not PROB_BF16:
    SCH_PIECES = set()  # int16 bit-pun needs the bf16 table container

_COMPILED = None  # cached nc across kernel() calls
_PREP_CACHE = {}  # host-prep results cached by input id


def _host_prep(x, relative_pos, Wq, bq, Wk, bk, Wv, bv, conv_w, conv_b,
               bn_gamma, bn_beta, bn_mean, bn_var, Wp, bp):
    """Fuse conv/BN into tap weights; fold biases; transpose activations."""
    import ml_dtypes
    f32 = np.float32
    bf16 = ml_dtypes.bfloat16
    wdt = bf16 if PROB_BF16 else f32
    x = np.asarray(x, f32)
    # xT: (B, C, N)
    xT = np.ascontiguousarray(x.transpose(0, 2, 1).astype(wdt))

    inv = (np.asarray(bn_gamma, f32)
           / np.sqrt(np.asarray(bn_var, f32) + BN_EPS))          # [c]
    wp_taps = np.asarray(conv_w, f32).reshape(C, SR * SR) * inv[:, None]  # [c,4]
    beta0 = (np.asarray(conv_b, f32) * inv
             + np.asarray(bn_beta, f32)
             - np.asarray(bn_mean, f32) * inv)                    # [c]

    Wk = np.asarray(Wk, f32)
    Wv = np.asarray(Wv, f32)
    # Wk_tap[t, c, c'] = wp_taps[c, t] * Wk[c, c']
    Wk_tap = np.ascontiguousarray(
        (wp_taps.T[:, :, None] * Wk[None, :, :]).astype(wdt))     # (4, C, C)
    Wv_tap = np.ascontiguousarray(
        (wp_taps.T[:, :, None] * Wv[None, :, :]).astype(wdt))

    # v bias (uniform over kv positions -> exact fold into final bias)
    beta_v = beta0 @ Wv + np.asarray(bv, f32)                     # [c']
    bp_col = (np.asarray(bp, f32) + beta_v @ np.asarray(Wp, f32)).reshape(C, 1)

    # scores are computed as S*K_SCH on device (Wq pre-scaled); the exact
    # pieces undo it inside the activation (scale=1/K_SCH), the schraudolph
    # pieces consume it directly.
    Wq_s = np.ascontiguousarray(
        (np.asarray(Wq, f32) * (SCALE * K_SCH)).astype(wdt))
    bq_col = (np.asarray(bq, f32) * (SCALE * K_SCH)).reshape(C, 1)

    # rel^T table interleaved per n-chunk: (NCH, C, 7, HEADS, NSL).
    # Exact pieces (r, hp) store exp(R) in bf16; schraudolph pieces store
    # int16(K*R + B - C) bit-punned into the same bf16 container.
    rel = np.asarray(relative_pos, f32)                  # (4, N, NKV)
    rT = rel.transpose(0, 2, 1)                          # (4, NKV, N) f32
    expI = np.zeros((NCH, C, 7, HEADS, NSL), wdt)
    for j, (m0, cnt) in enumerate(M_CHUNKS):
        # (4, cnt, NCH, NSL) -> (NCH, cnt, h, NSL)
        blk = rT[:, m0:m0 + cnt, :].reshape(HEADS, cnt, NCH, NSL)
        blk = blk.transpose(2, 1, 0, 3)                  # (NCH, cnt, h, NSL)
        for hp in range(2):
            part = blk[:, :, 2 * hp:2 * hp + 2, :]
            if (j, hp) in SCH_PIECES:
                t = np.round(part * K_SCH + (B_SCH - C_SCH))
                dst = t.astype(np.int16).view(wdt)
            else:
                dst = np.exp(part).astype(wdt)
            expI[:, 0:cnt, j, 2 * hp:2 * hp + 2, :] = dst
    expI = np.ascontiguousarray(expI)

    # concat all bf16 weights into one [C, 10C] tensor (one DMA):
    # wq | wk taps (4C, interleaved t-major per c) | wv taps | wp
    Wp_c = np.asarray(Wp, f32).astype(wdt)
    wall = np.concatenate(
        [Wq_s,
         Wk_tap.transpose(1, 0, 2).reshape(C, 4 * C),
         Wv_tap.transpose(1, 0, 2).reshape(C, 4 * C),
         Wp_c], axis=1)
    ball = np.concatenate([bq_col, bp_col], axis=1)  # [C, 2] f32

    return dict(xT=xT, Wall=np.ascontiguousarray(wall),
                ball=np.ascontiguousarray(ball), expI=expI)


def _build():
    """Build + compile the SPMD bass program (same NEFF for all 8 cores)."""
    import concourse.bass as bass
    import concourse.tile as tile
    from concourse import bacc, mybir
    from concourse.masks import make_identity

    f32 = mybir.dt.float32
    f32r = mybir.dt.float32r
    pdt = mybir.dt.bfloat16 if PROB_BF16 else f32

    nc = bacc.Bacc("TRN2", target_bir_lowering=False, debug=False,
                   num_devices=NCORES)

    # ---- DRAM I/O ----
    xT_d = nc.dram_tensor("xT", [C, N], pdt, kind="ExternalInput").ap()
    expI_d = nc.dram_tensor("expI", [NCH, C, 7 * HEADS * NSL], pdt,
                            kind="ExternalInput").ap()
    Wall_d = nc.dram_tensor("Wall", [C, 10 * C], pdt,
                            kind="ExternalInput").ap()
    ball_d = nc.dram_tensor("ball", [C, 2], f32, kind="ExternalInput").ap()
    out_d = nc.dram_tensor("out", [C, N], f32, kind="ExternalOutput").ap()

    with tile.TileContext(nc) as tc:
        from contextlib import ExitStack
        with ExitStack() as ctx:
            _emit(ctx, tc, nc, bass, mybir, make_identity, f32, f32r, pdt,
                  xT_d, expI_d, Wall_d, ball_d, out_d)

    nc.compile()
    return nc


def _emit(ctx, tc, nc, bass, mybir, make_identity, f32, f32r, pdt,
          xT_d, expI_d, Wall_d, ball_d, out_d):
    AF = mybir.ActivationFunctionType

    singles = ctx.enter_context(tc.tile_pool(name="singles", bufs=1))
    ppool = ctx.enter_context(tc.tile_pool(name="ppool", bufs=3))
    epool = ctx.enter_context(tc.tile_pool(name="epool", bufs=3))
    opool = ctx.enter_context(tc.tile_pool(name="opool", bufs=3))
    qpool = ctx.enter_context(tc.tile_pool(name="qpool", bufs=3))
    ptpool = ctx.enter_context(tc.tile_pool(name="ptpool", bufs=6))
    # PSUM: rot 3x2 + out 1 + rs 1 = 8 banks
    ps_rot = ctx.enter_context(tc.tile_pool(name="ps_rot", bufs=3,
                                            space="PSUM"))
    ps_out = ctx.enter_context(tc.tile_pool(name="ps_out", bufs=1,
                                            space="PSUM"))
    ps_rs = ctx.enter_context(tc.tile_pool(name="ps_rs", bufs=1,
                                           space="PSUM"))

    # ---- constants ----
    identb = singles.tile([C, C], pdt)
    make_identity(nc, identb[:])
    ones_sb = singles.tile([C, HD], pdt)
    nc.vector.memset(ones_sb[:], 1.0)

    # all weights in ONE DMA (the ~700ns/DMA sequencer issue cost was
    # serializing the fill); biases in a second small one.
    wall_sb = singles.tile([C, 10 * C], pdt)
    nc.scalar.dma_start(out=wall_sb[:], in_=Wall_d)
    # biases ride the (quiet) sync ring so their completion isn't lumped
    # behind the exp-table loads on the scalar ring
    ball_sb = singles.tile([C, 2], f32)
    nc.sync.dma_start(out=ball_sb[:], in_=ball_d)
    wq_sb = wall_sb[:, 0:C]
    wk_sb = wall_sb[:, C:5 * C].rearrange("p (t d) -> p t d", t=4)
    wv_sb = wall_sb[:, 5 * C:9 * C].rearrange("p (t d) -> p t d", t=4)
    wp_sb = wall_sb[:, 9 * C:10 * C]
    bq_sb = ball_sb[:, 0:1]
    bp_sb = ball_sb[:, 1:2]

    # whole-batch activations + k/v, resident all kernel
    xT_sb = singles.tile([C, N], pdt)
    nc.sync.dma_start(out=xT_sb[:, 0:N // 2], in_=xT_d[:, 0:N // 2])
    nc.sync.dma_start(out=xT_sb[:, N // 2:N], in_=xT_d[:, N // 2:N])
    kT_sb = singles.tile([C, 7 * 128], pdt)
    nc.vector.memset(kT_sb[:, NKV:7 * 128], 0.0)
    vT_sb = singles.tile([C, NKV], pdt)
    v_sb = singles.tile([C, 7, HEADS, HD], pdt)

    state = {}
    pp_of = {}
    exp_of = {}

    def prep_tap(which, mc):
        """One kv-chunk of the fused conv-tap projection (4 matmuls)."""
        dst = kT_sb if which == 0 else vT_sb
        w_sb = wk_sb if which == 0 else wv_sb
        xview = xT_sb[:].rearrange("p (i a j c) -> p a c i j",
                                   i=28, a=2, j=28, c=2)
        ps_kv = ps_rot.tile([C, 2, 512], f32, tag="rot", name="ps_kv")
        ps_kv = ps_kv[:, 0, :]
        for t in range(SR * SR):
            di, dj = t // 2, t % 2
            rhs = xview[:, di, dj, 14 * mc:14 * mc + 14, :]
            nc.tensor.matmul(ps_kv[:, 0:392], lhsT=w_sb[:, t, :],
                             rhs=rhs, start=(t == 0), stop=(t == 3))
        nc.vector.tensor_copy(dst[:, 392 * mc:392 * (mc + 1)], ps_kv[:, 0:392])

    def prep_vtrans(j):
        m0, cnt = M_CHUNKS[j]
        ps_t = ps_rot.tile([C, 2, 512], pdt, tag="rot", name="ps_t")
        ps_t = ps_t[:, 0, :]
        nc.tensor.transpose(ps_t[0:cnt, 0:C], vT_sb[:, m0:m0 + cnt],
                            identb[:])
        nc.vector.tensor_copy(
            v_sb[0:cnt, j, :, :],
            ps_t[0:cnt, 0:C].rearrange("p (h d) -> p h d", h=HEADS, d=HD))

    def exp_load(nch, eng=None):
        """Prefetch the exp(rel) interleave for slot nch (4 DMAs: finer
        completion granularity lets early pieces start before the whole
        2.8MB table lands)."""
        eng = eng if eng is not None else nc.sync
        e_sb = epool.tile([C, 7, HEADS, NSL], pdt, tag="expTI", name="e_sb")
        exp_of[nch] = e_sb
        flat = e_sb[:].rearrange("p a h n -> p (a h n)")
        tot = 7 * HEADS * NSL
        qtr = tot // 4
        for i in range(4):
            eng.dma_start(out=flat[:, i * qtr:(i + 1) * qtr],
                          in_=expI_d[nch, :, i * qtr:(i + 1) * qtr])

    qT_sb = singles.tile([C, N], pdt)

    def prep_q(qc):
        """Whole-batch q projection, one slot's columns at fill time."""
        ps_q = ps_rot.tile([C, 2, 512], f32, tag="rot", name="ps_q")
        c0 = qc * NSL
        nc.tensor.matmul(ps_q[:, 0, 0:NSL], lhsT=wq_sb,
                         rhs=xT_sb[:, c0:c0 + NSL],
                         start=True, stop=True)
        nc.scalar.activation(qT_sb[:, c0:c0 + NSL],
                             ps_q[:, 0, 0:NSL], AF.Identity, bias=bq_sb)

    sco_of = {}

    def scores_pair(g, hp):
        """One head-pair of round g's score matmuls (2-way row packing).
        Pair 1 waits on the previous round's schraudolph ADD releasing its
        psum tile; emitting it one step later than pair 0 lets the ready
        attn@v/rowsum groups run during that wait instead of queuing
        behind it in the strict-FIFO Tensor queue."""
        if g >= NCH * 7:
            return
        nch, r = g // 7, g % 7
        ps_s = ps_rot.tile([C, 2, 512], f32, tag="rot", name="ps_s")
        sco_of.setdefault(g, [None, None])[hp] = ps_s
        for h in (2 * hp, 2 * hp + 1):
            nc.tensor.matmul(
                ps_s[0:128, h % 2, 0:NSL],
                lhsT=kT_sb[HD * h:HD * (h + 1), 128 * r:128 * (r + 1)],
                rhs=qT_sb[HD * h:HD * (h + 1), nch * NSL:(nch + 1) * NSL],
                start=True, stop=True,
                tile_position=(HD * h, 0))

    def scores_round(g):
        scores_pair(g, 0)
        scores_pair(g, 1)

    i16 = mybir.dt.int16

    def exp_mul(nch, r, hp):
        """exp(S+R) for chunk r, head pair hp: either exact (ScalarE exp of
        S*K/K then DVE/GpSimd multiply by the exp(R) table) or fused
        schraudolph (one DVE add of the int16 R-table onto S*K in PSUM,
        int16 result bits == bf16 exp)."""
        g = nch * 7 + r
        ps_s = sco_of[g][hp]
        if hp == 1:
            del sco_of[g]
        pslice = pp_of[nch][:, r, 2 * hp:2 * hp + 2, :]
        eslice = exp_of[nch][:, r, 2 * hp:2 * hp + 2, :]
        if (r, hp) in SCH_PIECES:
            nc.vector.tensor_add(pslice.bitcast(i16), ps_s[:, :, 0:NSL],
                                 eslice.bitcast(i16))
            return
        pt_sb = ptpool.tile([C, 2 * NSL], pdt, tag="pt")
        nc.scalar.activation(pt_sb[:], ps_s[:, :, 0:NSL], AF.Exp,
                             scale=1.0 / K_SCH)
        eng = nc.gpsimd if (r, hp) in GPS_MULS else nc.vector
        off = (r * HEADS + 2 * hp) * NSL
        ppf = pp_of[nch][:].rearrange("p a h n -> p (a h n)")
        ef = exp_of[nch][:].rearrange("p a h n -> p (a h n)")
        eng.tensor_mul(ppf[:, off:off + 2 * NSL], pt_sb[:],
                       ef[:, off:off + 2 * NSL])

    def attnv4(nch, r, part=None):
        """attn@v + rowsums for kv chunk r: all 4 heads column-packed.
        part='av' emits only attn@v, 'rs' only rowsums (lets the two groups
        land on different pipeline steps). The last slot accumulates in a
        retired scores-rotation tile so its rounds can start before the
        previous slot's extract frees ov/z."""
        s = state[nch]
        m0, cnt = M_CHUNKS[r]
        if r == 0 and "ov" not in s:
            if nch == NCH - 1:
                ovz = ps_rot.tile([C, 2, 512], f32, tag="rot", name="ps_ovz")
                s["ov"] = ovz[:, 0, :]
                s["z"] = ovz[:, 1, :]
            else:
                s["ov"] = ps_out.tile([C, 512], f32, tag="out", name="ps_ov")
                s["z"] = ps_rs.tile([C, 512], f32, tag="rs", name="ps_z")
        ps_ov, ps_z = s["ov"], s["z"]
        pp = pp_of[nch]
        if part in (None, "av"):
            for h in range(HEADS):
                nc.tensor.matmul(
                    ps_ov[HD * h:HD * (h + 1), 0:NSL],
                    lhsT=v_sb[0:cnt, r, h, :],
                    rhs=pp[0:cnt, r, h, :],
                    start=(r == 0), stop=(r == len(M_CHUNKS) - 1),
                    tile_position=(0, HD * h), skip_group_check=True)
        if part in (None, "rs"):
            for h in range(HEADS):
                nc.tensor.matmul(
                    ps_z[HD * h:HD * (h + 1), 0:NSL],
                    lhsT=ones_sb[0:cnt, :],
                    rhs=pp[0:cnt, r, h, :],
                    start=(r == 0), stop=(r == len(M_CHUNKS) - 1),
                    tile_position=(0, HD * h), skip_group_check=True)

    def extract(nch):
        """Normalize straight out of PSUM: recip(rowsums), multiply."""
        s = state[nch]
        ps_ov = s.pop("ov")
        ps_z = s.pop("z")
        rb_sb = opool.tile([C, NSL], f32, tag="rb")
        nc.vector.reciprocal_approx_fast(rb_sb[:], ps_z[0:C, 0:NSL])
        outT_sb = opool.tile([C, NSL], pdt, tag="outT")
        s["outT"] = outT_sb
        nc.vector.tensor_mul(outT_sb[:], ps_ov[0:C, 0:NSL], rb_sb[:])

    def proj_tail(nch):
        """Final projection in transposed layout; host untransposes."""
        s = state[nch]
        ps_ft = ps_rot.tile([C, 2, 512], f32, tag="rot", name="ps_ft")
        ps_ft = ps_ft[:, 0, :]
        nc.tensor.matmul(ps_ft[0:C, 0:NSL], lhsT=wp_sb,
                         rhs=s.pop("outT")[:], start=True, stop=True)
        fin_sb = opool.tile([C, NSL], f32, tag="fin")
        # bias-add on ScalarE: DVE is loaded with schraudolph/mul work
        nc.scalar.activation(fin_sb[:], ps_ft[0:C, 0:NSL],
                             AF.Identity, bias=bp_sb)
        nc.sync.dma_start(out=out_d[:, nch * NSL:(nch + 1) * NSL],
                          in_=fin_sb[:])
        state.pop(nch)
        pp_of.pop(nch, None)
        exp_of.pop(nch, None)

    # ---- fill: k/v + all of q once, first exp tables ----
    # exp tables ride the sync ring: the scalar sequencer must stay free to
    # issue the q-bias IDENTITYs (ring backpressure from 2.8MB table DMAs
    # otherwise stalls everything queued behind them on that engine).
    exp_load(0, nc.sync)
    # scores_round(0) needs only kT chunk 0 (kv cols 0:128 -- inside the
    # first k-tap half) and the first q slot; everything else (second
    # k-tap half, v-taps, remaining q) comes off the critical path.
    prep_tap(0, 0)
    prep_q(0)
    scores_round(0)
    prep_tap(0, 1)
    for mc in range(2):
        prep_tap(1, mc)
    for qc in range(1, NCH):
        prep_q(qc)
    for j in range(7):
        prep_vtrans(j)
    exp_load(1, nc.sync)
    # ---- steady loop over n-chunk slots ----
    for nch in range(NCH):
        pp_of[nch] = ppool.tile([C, 7, HEADS, NSL], pdt, tag="pp",
                                name="pp_sb")
        state.setdefault(nch, {})
        for step in range(14):
            r, hp = step // 2, step % 2
            scores_pair(nch * 7 + r + 1, hp)
            exp_mul(nch, r, hp)
            if nch >= 1:
                if hp == 0 and step <= 12:
                    attnv4(nch - 1, step // 2)
                elif step == 13:
                    extract(nch - 1)
            if nch == NCH - 1 and step >= 12:
                # no more score rounds by now -- pack the last slot's attn@v
                # chunks into the PE-idle window while the final pieces run
                attnv4(nch, 2 * (step - 12))
                attnv4(nch, 2 * (step - 12) + 1)
            if nch >= 2 and step == 0:
                proj_tail(nch - 2)
            if step == 1 and nch + 2 < NCH:
                exp_load(nch + 2)
    # drain
    proj_tail(NCH - 2)
    for r in range(4, 7):
        attnv4(NCH - 1, r)
    extract(NCH - 1)
    proj_tail(NCH - 1)


def _get_compiled():
    global _COMPILED
    if _COMPILED is None:
        _COMPILED = _build()
    return _COMPILED


def make_in_map(prep, j):
    return {
        "xT": np.ascontiguousarray(prep["xT"][j]),
        "expI": prep["expI"].reshape(NCH, C, 7 * HEADS * NSL),
        "Wall": prep["Wall"], "ball": prep["ball"],
    }


def kernel(x, relative_pos, Wq, bq, Wk, bk, Wv, bv, conv_w, conv_b,
           bn_gamma, bn_beta, bn_mean, bn_var, Wp, bp, H=56, W=56,
           _trace=False):
    from concourse.bass_utils import run_bass_kernel_spmd

    prep = _host_prep(x, relative_pos, Wq, bq, Wk, bk, Wv, bv, conv_w,
                      conv_b, bn_gamma, bn_beta, bn_mean, bn_var, Wp, bp)
    nc = _get_compiled()

    in_maps = [make_in_map(prep, j) for j in range(NCORES)]

    res = run_bass_kernel_spmd(nc, in_maps, core_ids=list(range(NCORES)),
                               trace=_trace)

    out = np.empty((B, N, C), np.float32)
    for j in range(NCORES):
        out[j] = res.results[j]["out"].T
    if _trace:
        kernel._last_result = res
    return out

